# revision 44
# baseline (speedup 1.0000x reference)
"""Trainium2 Bass kernel for nn_MoELayer_5566277616585.

MoE layer with a quirk: the gate FFN outputs H=1024 logits, top-2 indices
>= E=8 are dropped, so ~98% of tokens route nowhere.  Strategy:

Launch 1 (bf16, fast): token-parallel gate FFN on 8 cores (512 tokens
  each; bf16 x@wi and h@wo).  Returns per-token top-8 logit slice +
  (max, 2nd max).  Approx error ~4.5e-2 on device, so launch 1 only
  *selects candidates* with a tau-margin superset.  mm2 runs
  token-block-outer, halves sequential, with per-half top-2 stats
  (m2 = max(min(m1a,m1b), m2a, m2b)) emitted inline so the stats tail
  overlaps remaining matmuls.
Launch 2: expert-parallel bf16 FFN over candidate tokens (core c =
  expert c) PLUS an F-sharded exact fp32 recompute of the gate logits for
  all candidate tokens (1/8 of ffn dim per core).  The recompute streams
  on the Act DMA queue and its mm2 is interleaved between the expert's
  mm1/mm2 so the PE stays fed while 17MB of expert weights stream on the
  SP queue.  Host combines: exact top-2 membership + exact weights from
  the recomputed logits.  (The recompute must stay fp32: the data's
  minimum real-expert decision gap is 5.9e-4, so f32r/bf16 would risk a
  membership flip.)

SELU is composed as  selu(z) = Relu(lam*z + lam*b) + lam*alpha*min(exp(z+b),1)
  - lam*alpha, with the constant -lam*alpha folded into the *output* bias
  via  bo_adj = bo - lam*alpha*colsum(wo).

All weight/activation tensors are pre-arranged on the host into
[128, chunk, subtile, cols] layouts so each launch needs only ~15 large
DMAs (HWDGE descriptor-generation overhead is ~625ns per DMA and was the
previous bottleneck at ~150 DMAs/launch).
"""

import numpy as np

import jax
from jax.experimental.shard_map import shard_map
from jax.sharding import Mesh, PartitionSpec

import concourse.bass as bass
import concourse.tile as tile
from concourse import bacc, mybir
from concourse.bass2jax import _bass_exec_p, install_neuronx_cc_hook, partition_id_tensor

F32 = mybir.dt.float32
F32R = mybir.dt.float32r
BF16 = mybir.dt.bfloat16
AX = mybir.AxisListType
OP = mybir.AluOpType
ACT = mybir.ActivationFunctionType

B, S, H, F, E = 2, 2048, 1024, 4096, 8
N = B * S              # 4096 tokens
NCORES = 8
TOK = N // NCORES      # 512 tokens per core in launch 1
TB = TOK // 128        # token blocks per core
LAM = 1.0507009873554805
ALPHA = 1.6732632423543772
LAM_ALPHA = LAM * ALPHA
TAU = 0.15             # candidate margin (bf16 gate err ~4.5e-2 on device)

HT = H // 128          # 8 h-subtiles (contraction tiles for mm1)
FT = F // 128          # 32 f-subtiles (contraction tiles for mm2)
FCH = 8                # wi/wo DMA chunks; each covers F//FCH = 512 f-cols
FPC = FT // FCH        # f-subtiles per chunk = 4
FS = F // NCORES       # 512: gate ffn shard per core in launch 2
FST = FS // 128        # 4 f-subtiles in the gate shard


def _ffn_bf16(nc, pools, xt_t, wi_ap, wo_ap, lbi, bi, boa, out_sb, ntok,
              on_block=None, xstride=None, pre_mm2=None):
    """out_sb[:ntok, :H] = selu'(x@wi+bi) @ wo + boa, all-bf16 matmuls.

    xt_t: SBUF [128, HT*ntok] bf16 (x transposed, h-subtile-major).
    wi_ap: DRAM [128, FCH, HT, F//FCH] bf16.  wo_ap: DRAM [128, FCH, FPC, H].
    lbi/bi: SBUF [128, FT] per-partition biases (lam*b, b).
    boa: SBUF [128, H] adjusted output bias (row-replicated) or None.
    mm2 runs token-block-outer; after block tc's output is written,
    on_block(tc) is invoked so the caller can emit dependent work early.
    """
    wipool, wopool, ps1, ps2, tpool, hpool = pools
    FW = F // FCH
    if xstride is None:
        xstride = ntok
    hs = []
    for fc in range(FCH):
        w = wipool.tile([128, HT * FW], BF16, tag="wi")
        nc.sync.dma_start(w[:], wi_ap[:, fc])
        for f4 in range(FPC):
            ft = fc * FPC + f4
            ps = ps1.tile([128, ntok], F32)
            for ht in range(HT):
                o = ht * FW + f4 * 128
                nc.tensor.matmul(ps[:], w[:, o:o + 128],
                                 xt_t[:, ht * xstride:ht * xstride + ntok],
                                 start=(ht == 0), stop=(ht == HT - 1))
            # selu'(z) = relu(lam*z + lam*b) + lam*alpha*min(exp(z+b), 1)
            r = tpool.tile([128, ntok], F32, tag="selu_r")
            nc.scalar.activation(r[:], ps[:], ACT.Relu,
                                 bias=lbi[:, ft:ft + 1], scale=LAM)
            t = tpool.tile([128, ntok], F32, tag="selu_t")
            nc.scalar.activation(t[:], ps[:], ACT.Exp,
                                 bias=bi[:, ft:ft + 1], scale=1.0)
            e2 = tpool.tile([128, ntok], F32, tag="selu_e")
            nc.vector.tensor_scalar(e2[:], t[:], 1.0, LAM_ALPHA,
                                    op0=OP.min, op1=OP.mult)
            h = hpool.tile([128, ntok], BF16, tag="h")
            nc.vector.tensor_tensor(h[:], r[:], e2[:], op=OP.add)
            hs.append(h)
    if pre_mm2 is not None:
        pre_mm2()
    # --- matmul 2: out[tok, H] = h @ wo (+ boa), token-block-outer ---
    tchunks = (ntok + 127) // 128
    wos = {}
    for tc in range(tchunks):
        tn = min(128, ntok - tc * 128)
        if on_block is None:
            # halves interleaved per ft (best for streaming wo chunks)
            pss = [ps2.tile([tn, 512], F32, tag="pss", name=f"pss{tc}_{hc}")
                   for hc in range(2)]
            for ft in range(FT):
                fc, f8 = ft // FPC, ft % FPC
                if tc == 0 and f8 == 0:
                    wt = wopool.tile([128, FPC * H], BF16, tag="wo")
                    nc.sync.dma_start(wt[:], wo_ap[:, fc])
                    wos[fc] = wt
                h_sl = hs[ft][:, tc * 128:tc * 128 + tn]
                for hc in range(2):
                    wo_sl = wos[fc][:, f8 * H + hc * 512:f8 * H + hc * 512 + 512]
                    nc.tensor.matmul(pss[hc][:], h_sl, wo_sl,
                                     start=(ft == 0), stop=(ft == FT - 1))
            for hc in range(2):
                dst = out_sb[tc * 128:tc * 128 + tn, hc * 512:hc * 512 + 512]
                if boa is not None:
                    nc.vector.tensor_add(dst, pss[hc][:],
                                         boa[:tn, hc * 512:hc * 512 + 512])
                else:
                    nc.vector.tensor_copy(dst, pss[hc][:])
        else:
            # halves sequential so the first half's dependents (on_block)
            # overlap the second half's matmuls
            for hc in range(2):
                psh = ps2.tile([tn, 512], F32, tag="pss",
                               name=f"pss{tc}_{hc}")
                for ft in range(FT):
                    fc, f8 = ft // FPC, ft % FPC
                    if tc == 0 and hc == 0 and f8 == 0:
                        wt = wopool.tile([128, FPC * H], BF16, tag="wo")
                        nc.sync.dma_start(wt[:], wo_ap[:, fc])
                        wos[fc] = wt
                    h_sl = hs[ft][:, tc * 128:tc * 128 + tn]
                    wo_sl = wos[fc][:, f8 * H + hc * 512:f8 * H + hc * 512 + 512]
                    nc.tensor.matmul(psh[:], h_sl, wo_sl,
                                     start=(ft == 0), stop=(ft == FT - 1))
                dst = out_sb[tc * 128:tc * 128 + tn, hc * 512:hc * 512 + 512]
                if boa is not None:
                    nc.vector.tensor_add(dst, psh[:],
                                         boa[:tn, hc * 512:hc * 512 + 512])
                else:
                    nc.vector.tensor_copy(dst, psh[:])
                on_block(tc, hc)
    return hs


class _RowView:
    """Token-major [ntok, hout] view over a list of [128, hout] tiles."""

    def __init__(self, tiles):
        self.tiles = tiles

    def __getitem__(self, idx):
        tokslice, hslice = idx
        tc0 = tokslice.start // 128
        return self.tiles[tc0][0:tokslice.stop - tokslice.start, hslice]


def build_gate_program(repeat=1):
    """Launch 1: gate FFN + top-2 stats for 512 tokens/core (all bf16)."""
    nc = bacc.Bacc("TRN2", target_bir_lowering=False, debug=False,
                   num_devices=NCORES)
    xt = nc.dram_tensor("xt", [128, HT, TOK], BF16, kind="ExternalInput").ap()
    gwi = nc.dram_tensor("gwi", [128, FCH, HT, F // FCH], BF16,
                         kind="ExternalInput").ap()
    gwo = nc.dram_tensor("gwo", [128, FCH, FPC, H], BF16,
                         kind="ExternalInput").ap()
    cst = nc.dram_tensor("cst", [128, 2 * FT + H], F32,
                         kind="ExternalInput").ap()
    st = nc.dram_tensor("st", [128, TB * 10], F32, kind="ExternalOutput").ap()

    with tile.TileContext(nc) as tc:
        import contextlib
        with contextlib.ExitStack() as ctx:
            xpool = ctx.enter_context(tc.tile_pool(name="x", bufs=1))
            cpool = ctx.enter_context(tc.tile_pool(name="consts", bufs=1))
            wipool = ctx.enter_context(tc.tile_pool(name="wi", bufs=3))
            wopool = ctx.enter_context(tc.tile_pool(name="wo", bufs=FCH))
            ps1 = ctx.enter_context(tc.tile_pool(name="ps1", bufs=4, space="PSUM"))
            ps2 = ctx.enter_context(tc.tile_pool(name="ps2", bufs=4, space="PSUM"))
            tpool = ctx.enter_context(tc.tile_pool(name="tmp", bufs=3))
            hpool = ctx.enter_context(tc.tile_pool(name="h", bufs=FT))
            zpool = ctx.enter_context(tc.tile_pool(name="z", bufs=TB))
            spool = ctx.enter_context(tc.tile_pool(name="small", bufs=8))
            epool = ctx.enter_context(tc.tile_pool(name="eq", bufs=2))

            def body(_i=None):
                xt_t = xpool.tile([128, HT * TOK], BF16, tag="xt")
                nc.scalar.dma_start(xt_t[:], xt[:, :, :])
                cst_t = cpool.tile([128, 2 * FT + H], F32, tag="cst")
                nc.scalar.dma_start(cst_t[:], cst[:, :])
                lbi = cst_t[:, 0:FT]
                bi = cst_t[:, FT:2 * FT]
                boa = cst_t[:, 2 * FT:2 * FT + H]

                zs = [zpool.tile([128, H], F32, tag="z", name=f"z{i}")
                      for i in range(TB)]
                stt = spool.tile([128, TB * 10], F32, tag="stt")
                halves = {}

                def stats_half(tcb, hc):
                    # per-half top-2; global m2 = max(min(m1a,m1b), m2a, m2b)
                    z = zs[tcb]
                    zh = z[:, hc * 512:(hc + 1) * 512]
                    m1h = spool.tile([128, 1], F32, tag="m1h",
                                     name=f"m1h{tcb}_{hc}")
                    nc.vector.tensor_reduce(m1h[:], zh, AX.X, OP.max)
                    eq = epool.tile([128, 512], F32, tag="eq")
                    nc.vector.tensor_scalar(eq[:], zh, m1h[:, 0:1], None,
                                            op0=OP.is_equal)
                    msk = epool.tile([128, 512], F32, tag="msk")
                    nc.vector.scalar_tensor_tensor(msk[:], eq[:], -1e30, zh,
                                                   op0=OP.mult, op1=OP.add)
                    m2h = spool.tile([128, 1], F32, tag="m2h",
                                     name=f"m2h{tcb}_{hc}")
                    nc.vector.tensor_reduce(m2h[:], msk[:], AX.X, OP.max)
                    halves[(tcb, hc)] = (m1h, m2h)
                    if hc == 1:
                        m1a, m2a = halves[(tcb, 0)]
                        m1b, m2b = halves[(tcb, 1)]
                        u = spool.tile([128, 1], F32, tag="u",
                                       name=f"u{tcb}")
                        nc.vector.tensor_tensor(u[:], m1a[:], m1b[:],
                                                op=OP.min)
                        v = spool.tile([128, 1], F32, tag="v",
                                       name=f"v{tcb}")
                        nc.vector.tensor_tensor(v[:], m2a[:], m2b[:],
                                                op=OP.max)
                        o = tcb * 10
                        nc.vector.tensor_copy(stt[:, o:o + E], z[:, 0:E])
                        nc.vector.tensor_tensor(stt[:, o + 8:o + 9],
                                                m1a[:], m1b[:], op=OP.max)
                        nc.vector.tensor_tensor(stt[:, o + 9:o + 10],
                                                u[:], v[:], op=OP.max)
                        nc.sync.dma_start(st[:, o:o + 10], stt[:, o:o + 10])

                _ffn_bf16(nc, (wipool, wopool, ps1, ps2, tpool, hpool),
                          xt_t, gwi, gwo, lbi, bi, boa, _RowView(zs), TOK,
                          on_block=stats_half)

            if repeat > 1:
                with tc.For_i(0, repeat, 1):
                    body()
            else:
                body()

    nc.compile()
    return nc


def build_ffn_program(ecap, ucap=128, repeat=1, parts="both"):
    """Launch 2: expert FFN on candidates (bf16) + exact gate F-shard (fp32)."""
    assert ecap <= 128 and ucap <= 512
    EC = max(ecap, 128)
    UB = (ucap + 127) // 128
    nc = bacc.Bacc("TRN2", target_bir_lowering=False, debug=False,
                   num_devices=NCORES)

    xct = nc.dram_tensor("xct", [128, HT, EC], BF16, kind="ExternalInput").ap()
    wi = nc.dram_tensor("wi", [128, FCH, HT, F // FCH], BF16,
                        kind="ExternalInput").ap()
    wo = nc.dram_tensor("wo", [128, FCH, FPC, H], BF16,
                        kind="ExternalInput").ap()
    cst = nc.dram_tensor("cst", [128, 2 * FT + H + 2 * FST], F32,
                         kind="ExternalInput").ap()
    xut = nc.dram_tensor("xut", [128, HT, ucap], F32, kind="ExternalInput").ap()
    gwis = nc.dram_tensor("gwis", [128, 2, HT, FS // 2], F32,
                          kind="ExternalInput").ap()
    gwos = nc.dram_tensor("gwos", [128, FST, H], F32, kind="ExternalInput").ap()
    y = nc.dram_tensor("y", [ecap, H], F32, kind="ExternalOutput").ap()
    gp = nc.dram_tensor("gp", [ucap, H], F32, kind="ExternalOutput").ap()

    with tile.TileContext(nc) as tc:
        import contextlib
        with contextlib.ExitStack() as ctx:
            xpool = ctx.enter_context(tc.tile_pool(name="x", bufs=2))
            cpool = ctx.enter_context(tc.tile_pool(name="consts", bufs=1))
            wipool = ctx.enter_context(tc.tile_pool(name="wi", bufs=2))
            wopool = ctx.enter_context(tc.tile_pool(name="wo", bufs=FCH))
            ps1 = ctx.enter_context(tc.tile_pool(name="ps1", bufs=3, space="PSUM"))
            ps2 = ctx.enter_context(tc.tile_pool(name="ps2", bufs=2, space="PSUM"))
            ps3 = ctx.enter_context(tc.tile_pool(name="ps3", bufs=2, space="PSUM"))
            tpool = ctx.enter_context(tc.tile_pool(name="tmp", bufs=3))
            hpool = ctx.enter_context(tc.tile_pool(name="h", bufs=FT + FST))
            opool = ctx.enter_context(tc.tile_pool(name="outs", bufs=2))

            def body(_i=None):
                do_expert = parts in ("both", "expert")
                do_shard = parts in ("both", "shard")
                cst_t = cpool.tile([128, 2 * FT + H + 2 * FST], F32, tag="cst")
                nc.sync.dma_start(cst_t[:], cst[:, :])
                lbi = cst_t[:, 0:FT]
                bi = cst_t[:, FT:2 * FT]
                boa = cst_t[:, 2 * FT:2 * FT + H]
                lgbis = cst_t[:, 2 * FT + H:2 * FT + H + FST]
                gbis = cst_t[:, 2 * FT + H + FST:2 * FT + H + 2 * FST]

                # gate F-shard exact fp32 recompute, emitted FIRST: its
                # inputs stream on the Act DMA queue while the (much larger)
                # expert weights stream concurrently on the SP queue, so the
                # recompute's matmuls fill the PE while expert weights load.
                if do_shard:
                    xut_t = xpool.tile([128, HT * ucap], F32, tag="xut")
                    nc.scalar.dma_start(xut_t[:], xut[:, :, :])
                    # gwis arrives in 2 chunks (cols split) so mm1 starts early
                    gwis_t = cpool.tile([128, HT * FS], F32, tag="gwis")
                    nc.scalar.dma_start(gwis_t[:, 0:HT * FS // 2],
                                        gwis[:, 0, :, :])
                    nc.scalar.dma_start(gwis_t[:, HT * FS // 2:],
                                        gwis[:, 1, :, :])
                    gwos_t = cpool.tile([128, FST * H], F32, tag="gwos")
                    nc.scalar.dma_start(gwos_t[:], gwos[:, :, :])
                    hus = []
                    for fst in range(FST):
                        ps = ps1.tile([128, ucap], F32)
                        ch, fl = fst // (FST // 2), fst % (FST // 2)
                        for ht in range(HT):
                            o = (ch * HT + ht) * (FS // 2) + fl * 128
                            nc.tensor.matmul(
                                ps[:], gwis_t[:, o:o + 128],
                                xut_t[:, ht * ucap:(ht + 1) * ucap],
                                start=(ht == 0), stop=(ht == HT - 1))
                        r = tpool.tile([128, ucap], F32, tag="selu_r")
                        nc.vector.tensor_scalar(r[:], ps[:],
                                                lgbis[:, fst:fst + 1], 0.0,
                                                op0=OP.add, op1=OP.max)
                        t = tpool.tile([128, ucap], F32, tag="selu_t")
                        nc.scalar.activation(t[:], ps[:], ACT.Exp,
                                             bias=gbis[:, fst:fst + 1],
                                             scale=1.0)
                        e2 = tpool.tile([128, ucap], F32, tag="selu_e")
                        nc.vector.tensor_scalar(e2[:], t[:], 1.0, LAM_ALPHA,
                                                op0=OP.min, op1=OP.mult)
                        hu = hpool.tile([128, ucap], F32, tag="hu")
                        nc.vector.scalar_tensor_tensor(hu[:], r[:], LAM, e2[:],
                                                       op0=OP.mult, op1=OP.add)
                        hus.append(hu)
                    def rec_mm2():
                        gps = [opool.tile([min(128, ucap - 128 * i), H], F32,
                                          tag="gp", name=f"gp{i}")
                               for i in range(UB)]
                        for hc in range(2):
                            pssu = [ps3.tile([min(128, ucap - 128 * i), 512],
                                             F32, tag="pssu",
                                             name=f"pssu{hc}_{i}")
                                    for i in range(UB)]
                            for fst in range(FST):
                                for i in range(UB):
                                    un = min(128, ucap - 128 * i)
                                    o = fst * H + hc * 512
                                    nc.tensor.matmul(
                                        pssu[i][:],
                                        hus[fst][:, i * 128:i * 128 + un],
                                        gwos_t[:, o:o + 512],
                                        start=(fst == 0),
                                        stop=(fst == FST - 1))
                            for i in range(UB):
                                nc.vector.tensor_copy(
                                    gps[i][:, hc * 512:hc * 512 + 512],
                                    pssu[i][:])
                        for i in range(UB):
                            un = min(128, ucap - 128 * i)
                            nc.scalar.dma_start(gp[128 * i:128 * i + un, :],
                                                gps[i][:])
                else:
                    rec_mm2 = None

                if do_expert:
                    xct_t = xpool.tile([128, HT * EC], BF16, tag="xct")
                    nc.sync.dma_start(xct_t[:], xct[:, :, :])
                    y_sb = opool.tile([ecap, H], F32, tag="y")
                    _ffn_bf16(nc, (wipool, wopool, ps1, ps2, tpool, hpool),
                              xct_t, wi, wo, lbi, bi, boa, y_sb[:, :], ecap,
                              xstride=EC, pre_mm2=rec_mm2)
                    nc.sync.dma_start(y[:, :], y_sb[:, :])
                elif rec_mm2 is not None:
                    rec_mm2()

            if repeat > 1:
                with tc.For_i(0, repeat, 1):
                    body()
            else:
                body()

    nc.compile()
    return nc


# ---------------------------------------------------------------------------
# SPMD runner (cached jit), mirrors concourse.bass2jax.run_bass_via_pjrt
# ---------------------------------------------------------------------------

def _build_runner(nc, n_cores=NCORES, donate=True):
    install_neuronx_cc_hook()
    partition_name = nc.partition_id_tensor.name if nc.partition_id_tensor else None
    in_names, out_names, out_avals, zero_shapes = [], [], [], []
    for alloc in nc.m.functions[0].allocations:
        if not isinstance(alloc, mybir.MemoryLocationSet):
            continue
        name = alloc.memorylocations[0].name
        if alloc.kind == "ExternalInput":
            if name != partition_name:
                in_names.append(name)
        elif alloc.kind == "ExternalOutput":
            out_names.append(name)
            shape = tuple(alloc.tensor_shape)
            dtype = mybir.dt.np(alloc.dtype)
            out_avals.append(jax.core.ShapedArray(shape, dtype))
            zero_shapes.append((shape, dtype))
    n_params = len(in_names)
    all_in_names = list(in_names) + list(out_names)
    if partition_name is not None:
        all_in_names.append(partition_name)
    donate_nums = tuple(range(n_params, n_params + len(out_names))) if donate else ()

    def _body(*args):
        operands = list(args)
        if partition_name is not None:
            operands.append(partition_id_tensor())
        return tuple(_bass_exec_p.bind(
            *operands,
            out_avals=tuple(out_avals),
            in_names=tuple(all_in_names),
            out_names=tuple(out_names),
            lowering_input_output_aliases=(),
            sim_require_finite=True,
            sim_require_nnan=True,
            nc=nc,
        ))

    devices = jax.devices()[:n_cores]
    mesh = Mesh(np.asarray(devices), ("core",))
    sharded = jax.jit(
        shard_map(_body, mesh=mesh,
                  in_specs=(PartitionSpec("core"),) * (n_params + len(out_names)),
                  out_specs=(PartitionSpec("core"),) * len(out_names),
                  check_rep=False),
        donate_argnums=donate_nums, keep_unused=True)

    def run(per_core_inputs):
        concat_in = [
            np.concatenate([np.ascontiguousarray(per_core_inputs[c][nm])
                            for c in range(n_cores)], axis=0)
            for nm in in_names
        ]
        concat_zeros = [np.zeros((n_cores * s[0], *s[1:]), d)
                        for s, d in zero_shapes]
        outs = sharded(*concat_in, *concat_zeros)
        return [
            {nm: np.asarray(outs[i]).reshape(n_cores, *out_avals[i].shape)[c]
             for i, nm in enumerate(out_names)}
            for c in range(n_cores)
        ]

    run.sharded = sharded
    run.in_names = in_names
    run.out_names = out_names
    run.out_avals = out_avals
    run.zero_shapes = zero_shapes
    run.mesh = mesh
    return run


_CACHE = {}


def _get_gate_runner():
    if "gate" not in _CACHE:
        _CACHE["gate"] = _build_runner(build_gate_program())
    return _CACHE["gate"]


def _get_ffn_runner(ecap, ucap):
    key = ("ffn", ecap, ucap)
    if key not in _CACHE:
        _CACHE[key] = _build_runner(build_ffn_program(ecap, ucap))
    return _CACHE[key]


# ---------------------------------------------------------------------------
# Host-side input prep (layout pre-arrangement; free wrt HW time)
# ---------------------------------------------------------------------------

def _bias_tiles(b, lam_scaled):
    """[F] -> [128, F//128]; column j = b[j*128:(j+1)*128] (tile-major)."""
    nb = (LAM * b if lam_scaled else b).astype(np.float32)
    return np.ascontiguousarray(nb.reshape(-1, 128).T)


def _sub3(a):
    """[H, W] -> [128, H//128, W]  (h-subtile-major transpose layout)."""
    Hh, W = a.shape
    return np.ascontiguousarray(a.reshape(Hh // 128, 128, W).transpose(1, 0, 2))


def _wi4(a):
    """wi [H, F] -> [128, FCH, HT, F//FCH]."""
    Hh, Ff = a.shape
    return np.ascontiguousarray(
        a.reshape(HT, 128, FCH, Ff // FCH).transpose(1, 2, 0, 3))


def _wo4(a):
    """wo [F, Hout] -> [128, FCH, FPC, Hout]."""
    Ff, Ho = a.shape
    return np.ascontiguousarray(
        a.reshape(FCH, FPC, 128, Ho).transpose(2, 0, 1, 3))


def _gwis2(a):
    """gate-wi shard [H, FS] -> [128, 2, HT, FS//2] (column-halved chunks)."""
    Hh, Fs = a.shape
    return np.ascontiguousarray(
        a.reshape(HT, 128, 2, Fs // 2).transpose(1, 2, 0, 3))


def _gate_cst(gate_bi, gate_wo, gate_bo):
    gbo_adj = (gate_bo.astype(np.float64)
               - LAM_ALPHA * gate_wo.astype(np.float64).sum(0)).astype(np.float32)
    return np.ascontiguousarray(np.concatenate([
        _bias_tiles(gate_bi, True), _bias_tiles(gate_bi, False),
        np.broadcast_to(gbo_adj, (128, H))], axis=1))


def _gate_inputs(xfT_b, gwi_b, gwo_b, cstg):
    """Per-core launch-1 inputs.  xfT_b: [H, N] bf16."""
    return [{
        "xt": _sub3(xfT_b[:, c * TOK:(c + 1) * TOK]),
        "gwi": gwi_b, "gwo": gwo_b, "cst": cstg,
    } for c in range(NCORES)]


def _parse_stats(res1):
    """-> l8 [N, 8] f32, m2 [N] f32 from per-core st outputs."""
    l8 = np.empty((N, E), np.float32)
    m2 = np.empty(N, np.float32)
    for c in range(NCORES):
        stc = res1[c]["st"]                      # [128, TB*10]
        for tcb in range(TB):
            rows = slice(c * TOK + tcb * 128, c * TOK + (tcb + 1) * 128)
            l8[rows] = stc[:, tcb * 10:tcb * 10 + 8]
            m2[rows] = stc[:, tcb * 10 + 9]
    return l8, m2


def kernel(x, gate_wi, gate_bi, gate_wo, gate_bo,
           exp_wi, exp_bi, exp_wo, exp_bo):
    import ml_dtypes
    _bf = ml_dtypes.bfloat16
    x = np.asarray(x, np.float32)
    gate_wi = np.asarray(gate_wi, np.float32)
    gate_bi = np.asarray(gate_bi, np.float32)
    gate_wo = np.asarray(gate_wo, np.float32)
    gate_bo = np.asarray(gate_bo, np.float32)
    exp_wi = np.asarray(exp_wi, np.float32)
    exp_bi = np.asarray(exp_bi, np.float32)
    exp_wo = np.asarray(exp_wo, np.float32)
    exp_bo = np.asarray(exp_bo, np.float32)

    xf = x.reshape(N, H)
    xfT = np.ascontiguousarray(xf.T)             # [H, N]
    xfT_b = xfT.astype(_bf)

    # ---- launch 1: gate + top-2 stats ----
    run1 = _get_gate_runner()
    ins1 = _gate_inputs(xfT_b, _wi4(gate_wi.astype(_bf)),
                        _wo4(gate_wo.astype(_bf)),
                        _gate_cst(gate_bi, gate_wo, gate_bo))
    res1 = run1(ins1)
    l8, m2a = _parse_stats(res1)

    # ---- candidate selection (superset, tau-margin) ----
    cand = l8 >= (m2a[:, None] - TAU)                    # [N, E]
    cand_tok = np.nonzero(cand.any(1))[0]
    nu = len(cand_tok)
    if nu == 0:
        return np.zeros((B, S, H), np.float32)
    ucap = 128
    while nu > ucap:
        ucap *= 2
    expert_rows = [np.nonzero(cand[:, e])[0] for e in range(E)]
    maxc = max((len(r) for r in expert_rows), default=1)
    ecap = 32
    while maxc > ecap:
        ecap *= 2

    # ---- launch 2: expert FFNs + exact gate recompute ----
    run2 = _get_ffn_runner(ecap, ucap)
    EC = max(ecap, 128)
    xut = np.zeros((H, ucap), np.float32)
    xut[:, :nu] = xfT[:, cand_tok]
    xut3 = _sub3(xut)
    ins2 = []
    for c in range(NCORES):
        rows = expert_rows[c]
        xct = np.zeros((H, EC), _bf)
        xct[:, :len(rows)] = xfT_b[:, rows]
        boa_c = (exp_bo[c].astype(np.float64)
                 - LAM_ALPHA * exp_wo[c].astype(np.float64).sum(0)).astype(np.float32)
        sl = slice(c * FS, (c + 1) * FS)
        cstf = np.ascontiguousarray(np.concatenate([
            _bias_tiles(exp_bi[c], True), _bias_tiles(exp_bi[c], False),
            np.broadcast_to(boa_c, (128, H)),
            _bias_tiles(LAM * gate_bi[sl], False),
            _bias_tiles(gate_bi[sl], False)], axis=1))
        ins2.append({
            "xct": _sub3(xct),
            "wi": _wi4(exp_wi[c].astype(_bf)),
            "wo": _wo4(exp_wo[c].astype(_bf)),
            "cst": cstf,
            "xut": xut3,
            "gwis": _gwis2(np.ascontiguousarray(gate_wi[:, sl])),
            "gwos": _sub3(np.ascontiguousarray(gate_wo[sl, :])),
        })
    res2 = run2(ins2)

    # ---- exact logits for candidate tokens, exact top-2 + weights ----
    gsum = np.zeros((ucap, H), np.float64)
    for c in range(NCORES):
        gsum += res2[c]["gp"].astype(np.float64)
    logits_u = gsum[:nu] + (gate_bo.astype(np.float64)
                            - LAM_ALPHA * gate_wo.astype(np.float64).sum(0))
    part = np.partition(logits_u, (H - 2, H - 1), axis=1)
    m1x, m2x = part[:, -1], part[:, -2]
    denom = m1x + m2x
    upos = np.full(N, -1, np.int64)
    upos[cand_tok] = np.arange(nu)

    out = np.zeros((N, H), np.float64)
    for e in range(E):
        rows = expert_rows[e]
        if len(rows) == 0:
            continue
        ye = res2[e]["y"][:len(rows)].astype(np.float64)
        pu = upos[rows]
        le = logits_u[pu, e]
        routed = le >= m2x[pu]
        wgt = np.where(routed, le / denom[pu], 0.0)
        out[rows] += wgt[:, None] * ye
    return out.reshape(B, S, H).astype(np.float32)


# revision 51
# speedup vs baseline: 1.2890x; 1.2890x over previous
"""Trainium2 Bass kernel for nn_MoELayer_5566277616585.

MoE layer with a quirk: the gate FFN outputs H=1024 logits, top-2 indices
>= E=8 are dropped, so ~98% of tokens route nowhere.  Strategy:

Launch 1 (bf16, fast): token-parallel gate FFN on 8 cores (512 tokens
  each; bf16 x@wi and h@wo).  Returns per-token top-8 logit slice +
  (max, 2nd max).  Approx error ~4.5e-2 on device, so launch 1 only
  *selects candidates* with a tau-margin superset.  mm2 runs
  token-block-outer, halves sequential, with per-half top-2 stats
  (m2 = max(min(m1a,m1b), m2a, m2b)) emitted inline so the stats tail
  overlaps remaining matmuls.
Launch 2: expert-parallel bf16 FFN over candidate tokens (core c =
  expert c) PLUS an F-sharded exact fp32 recompute of the gate logits for
  all candidate tokens (1/8 of ffn dim per core).  The recompute streams
  on the Act DMA queue and its mm2 is interleaved between the expert's
  mm1/mm2 so the PE stays fed while 17MB of expert weights stream on the
  SP queue.  Host combines: exact top-2 membership + exact weights from
  the recomputed logits.  (The recompute must stay fp32: the data's
  minimum real-expert decision gap is 5.9e-4, so f32r/bf16 would risk a
  membership flip.)

SELU is composed as  selu(z) = Relu(lam*z + lam*b) + lam*alpha*min(exp(z+b),1)
  - lam*alpha, with the constant -lam*alpha folded into the *output* bias
  via  bo_adj = bo - lam*alpha*colsum(wo).

All weight/activation tensors are pre-arranged on the host into
[128, chunk, subtile, cols] layouts so each launch needs only ~15 large
DMAs (HWDGE descriptor-generation overhead is ~625ns per DMA and was the
previous bottleneck at ~150 DMAs/launch).
"""

import numpy as np

import jax
from jax.experimental.shard_map import shard_map
from jax.sharding import Mesh, PartitionSpec

import concourse.bass as bass
import concourse.tile as tile
from concourse import bacc, mybir
from concourse.bass2jax import _bass_exec_p, install_neuronx_cc_hook, partition_id_tensor

F32 = mybir.dt.float32
F32R = mybir.dt.float32r
BF16 = mybir.dt.bfloat16
AX = mybir.AxisListType
OP = mybir.AluOpType
ACT = mybir.ActivationFunctionType

B, S, H, F, E = 2, 2048, 1024, 4096, 8
N = B * S              # 4096 tokens
NCORES = 8
TOK = N // NCORES      # 512 tokens per core in launch 1
TB = TOK // 128        # token blocks per core
LAM = 1.0507009873554805
ALPHA = 1.6732632423543772
LAM_ALPHA = LAM * ALPHA
TAU = 0.15             # candidate margin (bf16 gate err ~4.5e-2 on device)

HT = H // 128          # 8 h-subtiles (contraction tiles for mm1)
FT = F // 128          # 32 f-subtiles (contraction tiles for mm2)
FCH = 8                # wi/wo DMA chunks; each covers F//FCH = 512 f-cols
FPC = FT // FCH        # f-subtiles per chunk = 4
FS = F // NCORES       # 512: gate ffn shard per core in launch 2
FST = FS // 128        # 4 f-subtiles in the gate shard


def _ffn_bf16(nc, pools, xsl, wi_ap, wo_ap, lbi, bi, boa, out_sb, ntok,
              on_block=None, pre_mm2=None, mm2_fill=None):
    """out_sb[:ntok, :H] = selu'(x@wi+bi) @ wo + boa, all-bf16 matmuls.

    xsl(ht): returns the SBUF AP [128, ntok] of x^T for h-subtile ht.
    wi_ap: DRAM [128, FCH, HT, F//FCH] bf16.  wo_ap: DRAM [128, FCH, FPC, H].
    lbi/bi: SBUF [128, FT] per-partition biases (lam*b, b).
    boa: SBUF [128, H] adjusted output bias (row-replicated) or None.
    mm2 runs token-block-outer; after block tc's output is written,
    on_block(tc) is invoked so the caller can emit dependent work early.
    mm2_fill: list of thunks emitted at mm2 wo-chunk boundaries (PE filler
    while the next weight chunk streams).
    """
    wipool, wopool, ps1, ps2, tpool, hpool = pools
    FW = F // FCH
    hs = []
    for fc in range(FCH):
        w = wipool.tile([128, HT * FW], BF16, tag="wi")
        nc.sync.dma_start(w[:], wi_ap[:, fc])
        for f4 in range(FPC):
            ft = fc * FPC + f4
            ps = ps1.tile([128, ntok], F32)
            for ht in range(HT):
                o = ht * FW + f4 * 128
                nc.tensor.matmul(ps[:], w[:, o:o + 128], xsl(ht),
                                 start=(ht == 0), stop=(ht == HT - 1))
            # selu'(z) = relu(lam*z + lam*b) + lam*alpha*min(exp(z+b), 1)
            r = tpool.tile([128, ntok], F32, tag="selu_r")
            nc.scalar.activation(r[:], ps[:], ACT.Relu,
                                 bias=lbi[:, ft:ft + 1], scale=LAM)
            t = tpool.tile([128, ntok], F32, tag="selu_t")
            nc.scalar.activation(t[:], ps[:], ACT.Exp,
                                 bias=bi[:, ft:ft + 1], scale=1.0)
            e2 = tpool.tile([128, ntok], F32, tag="selu_e")
            nc.vector.tensor_scalar(e2[:], t[:], 1.0, LAM_ALPHA,
                                    op0=OP.min, op1=OP.mult)
            h = hpool.tile([128, ntok], BF16, tag="h")
            nc.vector.tensor_tensor(h[:], r[:], e2[:], op=OP.add)
            hs.append(h)
    if pre_mm2 is not None:
        pre_mm2()
    # --- matmul 2: out[tok, H] = h @ wo (+ boa), token-block-outer ---
    tchunks = (ntok + 127) // 128
    wos = {}
    for tc in range(tchunks):
        tn = min(128, ntok - tc * 128)
        if on_block is None:
            # halves interleaved per ft (best for streaming wo chunks)
            fills = list(mm2_fill or [])
            pss = [ps2.tile([tn, 512], F32, tag="pss", name=f"pss{tc}_{hc}")
                   for hc in range(2)]
            for ft in range(FT):
                fc, f8 = ft // FPC, ft % FPC
                if tc == 0 and f8 == 0:
                    wt = wopool.tile([128, FPC * H], BF16, tag="wo")
                    nc.sync.dma_start(wt[:], wo_ap[:, fc])
                    wos[fc] = wt
                h_sl = hs[ft][:, tc * 128:tc * 128 + tn]
                for hc in range(2):
                    wo_sl = wos[fc][:, f8 * H + hc * 512:f8 * H + hc * 512 + 512]
                    nc.tensor.matmul(pss[hc][:], h_sl, wo_sl,
                                     start=(ft == 0), stop=(ft == FT - 1))
                if tc == 0 and f8 == FPC - 1 and fills:
                    fills.pop(0)()
            for f in fills:
                f()
            for hc in range(2):
                dst = out_sb[tc * 128:tc * 128 + tn, hc * 512:hc * 512 + 512]
                if boa is not None:
                    nc.vector.tensor_add(dst, pss[hc][:],
                                         boa[:tn, hc * 512:hc * 512 + 512])
                else:
                    nc.vector.tensor_copy(dst, pss[hc][:])
        else:
            # halves sequential so the first half's dependents (on_block)
            # overlap the second half's matmuls
            for hc in range(2):
                psh = ps2.tile([tn, 512], F32, tag="pss",
                               name=f"pss{tc}_{hc}")
                for ft in range(FT):
                    fc, f8 = ft // FPC, ft % FPC
                    if tc == 0 and hc == 0 and f8 == 0:
                        wt = wopool.tile([128, FPC * H], BF16, tag="wo")
                        nc.sync.dma_start(wt[:], wo_ap[:, fc])
                        wos[fc] = wt
                    h_sl = hs[ft][:, tc * 128:tc * 128 + tn]
                    wo_sl = wos[fc][:, f8 * H + hc * 512:f8 * H + hc * 512 + 512]
                    nc.tensor.matmul(psh[:], h_sl, wo_sl,
                                     start=(ft == 0), stop=(ft == FT - 1))
                dst = out_sb[tc * 128:tc * 128 + tn, hc * 512:hc * 512 + 512]
                if boa is not None:
                    nc.vector.tensor_add(dst, psh[:],
                                         boa[:tn, hc * 512:hc * 512 + 512])
                else:
                    nc.vector.tensor_copy(dst, psh[:])
                on_block(tc, hc)
    return hs


class _RowView:
    """Token-major [ntok, hout] view over a list of [128, hout] tiles."""

    def __init__(self, tiles):
        self.tiles = tiles

    def __getitem__(self, idx):
        tokslice, hslice = idx
        tc0 = tokslice.start // 128
        return self.tiles[tc0][0:tokslice.stop - tokslice.start, hslice]


def build_gate_program(repeat=1):
    """Launch 1: gate FFN + top-2 stats for 512 tokens/core (all bf16)."""
    nc = bacc.Bacc("TRN2", target_bir_lowering=False, debug=False,
                   num_devices=NCORES)
    xt = nc.dram_tensor("xt", [128, HT, TOK], BF16, kind="ExternalInput").ap()
    gwi = nc.dram_tensor("gwi", [128, FCH, HT, F // FCH], BF16,
                         kind="ExternalInput").ap()
    gwo = nc.dram_tensor("gwo", [128, FCH, FPC, H], BF16,
                         kind="ExternalInput").ap()
    cst = nc.dram_tensor("cst", [128, 2 * FT + H], F32,
                         kind="ExternalInput").ap()
    st = nc.dram_tensor("st", [128, TB * 10], F32, kind="ExternalOutput").ap()

    with tile.TileContext(nc) as tc:
        import contextlib
        with contextlib.ExitStack() as ctx:
            xpool = ctx.enter_context(tc.tile_pool(name="x", bufs=1))
            cpool = ctx.enter_context(tc.tile_pool(name="consts", bufs=1))
            wipool = ctx.enter_context(tc.tile_pool(name="wi", bufs=3))
            wopool = ctx.enter_context(tc.tile_pool(name="wo", bufs=FCH))
            ps1 = ctx.enter_context(tc.tile_pool(name="ps1", bufs=4, space="PSUM"))
            ps2 = ctx.enter_context(tc.tile_pool(name="ps2", bufs=4, space="PSUM"))
            tpool = ctx.enter_context(tc.tile_pool(name="tmp", bufs=3))
            hpool = ctx.enter_context(tc.tile_pool(name="h", bufs=FT))
            zpool = ctx.enter_context(tc.tile_pool(name="z", bufs=TB))
            spool = ctx.enter_context(tc.tile_pool(name="small", bufs=8))
            epool = ctx.enter_context(tc.tile_pool(name="eq", bufs=2))

            def body(_i=None):
                # x^T in two tiles so mm1's first matmuls start after half
                xt_a = xpool.tile([128, HT // 2 * TOK], BF16, tag="xta")
                nc.scalar.dma_start(xt_a[:], xt[:, 0:HT // 2, :])
                xt_b = xpool.tile([128, HT // 2 * TOK], BF16, tag="xtb")
                nc.scalar.dma_start(xt_b[:], xt[:, HT // 2:, :])

                def xsl(ht):
                    t = xt_a if ht < HT // 2 else xt_b
                    o = (ht % (HT // 2)) * TOK
                    return t[:, o:o + TOK]

                cst_t = cpool.tile([128, 2 * FT + H], F32, tag="cst")
                nc.scalar.dma_start(cst_t[:], cst[:, :])
                lbi = cst_t[:, 0:FT]
                bi = cst_t[:, FT:2 * FT]
                boa = cst_t[:, 2 * FT:2 * FT + H]

                zs = [zpool.tile([128, H], F32, tag="z", name=f"z{i}")
                      for i in range(TB)]
                stt = spool.tile([128, TB * 10], F32, tag="stt")
                halves = {}

                def stats_half(tcb, hc):
                    # per-half top-2; global m2 = max(min(m1a,m1b), m2a, m2b)
                    z = zs[tcb]
                    zh = z[:, hc * 512:(hc + 1) * 512]
                    m1h = spool.tile([128, 1], F32, tag="m1h",
                                     name=f"m1h{tcb}_{hc}")
                    nc.vector.tensor_reduce(m1h[:], zh, AX.X, OP.max)
                    eq = epool.tile([128, 512], F32, tag="eq")
                    nc.vector.tensor_scalar(eq[:], zh, m1h[:, 0:1], None,
                                            op0=OP.is_equal)
                    msk = epool.tile([128, 512], F32, tag="msk")
                    nc.vector.scalar_tensor_tensor(msk[:], eq[:], -1e30, zh,
                                                   op0=OP.mult, op1=OP.add)
                    m2h = spool.tile([128, 1], F32, tag="m2h",
                                     name=f"m2h{tcb}_{hc}")
                    nc.vector.tensor_reduce(m2h[:], msk[:], AX.X, OP.max)
                    halves[(tcb, hc)] = (m1h, m2h)
                    if hc == 1:
                        m1a, m2a = halves[(tcb, 0)]
                        m1b, m2b = halves[(tcb, 1)]
                        u = spool.tile([128, 1], F32, tag="u",
                                       name=f"u{tcb}")
                        nc.vector.tensor_tensor(u[:], m1a[:], m1b[:],
                                                op=OP.min)
                        v = spool.tile([128, 1], F32, tag="v",
                                       name=f"v{tcb}")
                        nc.vector.tensor_tensor(v[:], m2a[:], m2b[:],
                                                op=OP.max)
                        o = tcb * 10
                        nc.vector.tensor_copy(stt[:, o:o + E], z[:, 0:E])
                        nc.vector.tensor_tensor(stt[:, o + 8:o + 9],
                                                m1a[:], m1b[:], op=OP.max)
                        nc.vector.tensor_tensor(stt[:, o + 9:o + 10],
                                                u[:], v[:], op=OP.max)
                        nc.sync.dma_start(st[:, o:o + 10], stt[:, o:o + 10])

                _ffn_bf16(nc, (wipool, wopool, ps1, ps2, tpool, hpool),
                          xsl, gwi, gwo, lbi, bi, boa, _RowView(zs), TOK,
                          on_block=stats_half)

            if repeat > 1:
                with tc.For_i(0, repeat, 1):
                    body()
            else:
                body()

    nc.compile()
    return nc


def build_ffn_program(ecap, ucap=128, repeat=1, parts="both"):
    """Launch 2: expert FFN on candidates (bf16) + exact gate F-shard (fp32)."""
    assert ecap <= 128 and ucap <= 512
    EC = max(ecap, 128)
    UB = (ucap + 127) // 128
    nc = bacc.Bacc("TRN2", target_bir_lowering=False, debug=False,
                   num_devices=NCORES)

    xct = nc.dram_tensor("xct", [128, HT, EC], BF16, kind="ExternalInput").ap()
    wi = nc.dram_tensor("wi", [128, FCH, HT, F // FCH], BF16,
                        kind="ExternalInput").ap()
    wo = nc.dram_tensor("wo", [128, FCH, FPC, H], BF16,
                        kind="ExternalInput").ap()
    cst = nc.dram_tensor("cst", [128, 2 * FT + H + 2 * FST], F32,
                         kind="ExternalInput").ap()
    xut = nc.dram_tensor("xut", [128, HT, ucap], F32, kind="ExternalInput").ap()
    gwis = nc.dram_tensor("gwis", [128, 2, HT, FS // 2], F32,
                          kind="ExternalInput").ap()
    gwos = nc.dram_tensor("gwos", [128, FST, H], F32, kind="ExternalInput").ap()
    y = nc.dram_tensor("y", [ecap, H], F32, kind="ExternalOutput").ap()
    gp = nc.dram_tensor("gp", [ucap, H], F32, kind="ExternalOutput").ap()

    with tile.TileContext(nc) as tc:
        import contextlib
        with contextlib.ExitStack() as ctx:
            xpool = ctx.enter_context(tc.tile_pool(name="x", bufs=2))
            cpool = ctx.enter_context(tc.tile_pool(name="consts", bufs=1))
            wipool = ctx.enter_context(tc.tile_pool(name="wi", bufs=2))
            wopool = ctx.enter_context(tc.tile_pool(name="wo", bufs=FCH))
            ps1 = ctx.enter_context(tc.tile_pool(name="ps1", bufs=3, space="PSUM"))
            ps2 = ctx.enter_context(tc.tile_pool(name="ps2", bufs=2, space="PSUM"))
            ps3 = ctx.enter_context(tc.tile_pool(name="ps3", bufs=2, space="PSUM"))
            tpool = ctx.enter_context(tc.tile_pool(name="tmp", bufs=3))
            hpool = ctx.enter_context(tc.tile_pool(name="h", bufs=FT + FST))
            opool = ctx.enter_context(tc.tile_pool(name="outs", bufs=2))

            def body(_i=None):
                do_expert = parts in ("both", "expert")
                do_shard = parts in ("both", "shard")
                cst_t = cpool.tile([128, 2 * FT + H + 2 * FST], F32, tag="cst")
                nc.sync.dma_start(cst_t[:], cst[:, :])
                lbi = cst_t[:, 0:FT]
                bi = cst_t[:, FT:2 * FT]
                boa = cst_t[:, 2 * FT:2 * FT + H]
                lgbis = cst_t[:, 2 * FT + H:2 * FT + H + FST]
                gbis = cst_t[:, 2 * FT + H + FST:2 * FT + H + 2 * FST]

                # gate F-shard exact fp32 recompute, emitted FIRST: its
                # inputs stream on the Act DMA queue while the (much larger)
                # expert weights stream concurrently on the SP queue, so the
                # recompute's matmuls fill the PE while expert weights load.
                if do_shard:
                    xut_t = xpool.tile([128, HT * ucap], F32, tag="xut")
                    nc.scalar.dma_start(xut_t[:], xut[:, :, :])
                    # gwis arrives in 2 chunks (cols split) so mm1 starts early
                    gwis_t = cpool.tile([128, HT * FS], F32, tag="gwis")
                    nc.scalar.dma_start(gwis_t[:, 0:HT * FS // 2],
                                        gwis[:, 0, :, :])
                    nc.scalar.dma_start(gwis_t[:, HT * FS // 2:],
                                        gwis[:, 1, :, :])
                    gwos_t = cpool.tile([128, FST * H], F32, tag="gwos")
                    nc.scalar.dma_start(gwos_t[:], gwos[:, :, :])
                    hus = []
                    for fst in range(FST):
                        ps = ps1.tile([128, ucap], F32)
                        ch, fl = fst // (FST // 2), fst % (FST // 2)
                        for ht in range(HT):
                            o = (ch * HT + ht) * (FS // 2) + fl * 128
                            nc.tensor.matmul(
                                ps[:], gwis_t[:, o:o + 128],
                                xut_t[:, ht * ucap:(ht + 1) * ucap],
                                start=(ht == 0), stop=(ht == HT - 1))
                        r = tpool.tile([128, ucap], F32, tag="selu_r")
                        nc.vector.tensor_scalar(r[:], ps[:],
                                                lgbis[:, fst:fst + 1], 0.0,
                                                op0=OP.add, op1=OP.max)
                        t = tpool.tile([128, ucap], F32, tag="selu_t")
                        nc.scalar.activation(t[:], ps[:], ACT.Exp,
                                             bias=gbis[:, fst:fst + 1],
                                             scale=1.0)
                        e2 = tpool.tile([128, ucap], F32, tag="selu_e")
                        nc.vector.tensor_scalar(e2[:], t[:], 1.0, LAM_ALPHA,
                                                op0=OP.min, op1=OP.mult)
                        hu = hpool.tile([128, ucap], F32, tag="hu")
                        nc.vector.scalar_tensor_tensor(hu[:], r[:], LAM, e2[:],
                                                       op0=OP.mult, op1=OP.add)
                        hus.append(hu)
                    # rec-mm2 split into 4 pieces, used as PE filler at the
                    # expert mm2's wo-chunk boundaries
                    gps = [opool.tile([min(128, ucap - 128 * i), H], F32,
                                      tag="gp", name=f"gp{i}")
                           for i in range(UB)]
                    pssus = {}

                    def rec_piece(hc, half):
                        def f():
                            if half == 0:
                                pssus[hc] = [
                                    ps3.tile([min(128, ucap - 128 * i), 512],
                                             F32, tag="pssu",
                                             name=f"pssu{hc}_{i}")
                                    for i in range(UB)]
                            pssu = pssus[hc]
                            for fst in ((0, 1) if half == 0 else (2, 3)):
                                for i in range(UB):
                                    un = min(128, ucap - 128 * i)
                                    o = fst * H + hc * 512
                                    nc.tensor.matmul(
                                        pssu[i][:],
                                        hus[fst][:, i * 128:i * 128 + un],
                                        gwos_t[:, o:o + 512],
                                        start=(fst == 0),
                                        stop=(fst == FST - 1))
                            if half == 1:
                                for i in range(UB):
                                    nc.vector.tensor_copy(
                                        gps[i][:, hc * 512:hc * 512 + 512],
                                        pssu[i][:])
                                if hc == 1:
                                    for i in range(UB):
                                        un = min(128, ucap - 128 * i)
                                        nc.scalar.dma_start(
                                            gp[128 * i:128 * i + un, :],
                                            gps[i][:])
                        return f

                    rec_pieces = [rec_piece(0, 0), rec_piece(0, 1),
                                  rec_piece(1, 0), rec_piece(1, 1)]
                else:
                    rec_pieces = None

                if do_expert:
                    xct_t = xpool.tile([128, HT * EC], BF16, tag="xct")
                    nc.sync.dma_start(xct_t[:], xct[:, :, :])

                    def xsl_e(ht):
                        return xct_t[:, ht * EC:ht * EC + ecap]

                    y_sb = opool.tile([ecap, H], F32, tag="y")
                    _ffn_bf16(nc, (wipool, wopool, ps1, ps2, tpool, hpool),
                              xsl_e, wi, wo, lbi, bi, boa, y_sb[:, :], ecap,
                              mm2_fill=rec_pieces)
                    nc.sync.dma_start(y[:, :], y_sb[:, :])
                elif rec_pieces is not None:
                    for f in rec_pieces:
                        f()

            if repeat > 1:
                with tc.For_i(0, repeat, 1):
                    body()
            else:
                body()

    nc.compile()
    return nc


# ---------------------------------------------------------------------------
# SPMD runner (cached jit), mirrors concourse.bass2jax.run_bass_via_pjrt
# ---------------------------------------------------------------------------

def _build_runner(nc, n_cores=NCORES, donate=True):
    install_neuronx_cc_hook()
    partition_name = nc.partition_id_tensor.name if nc.partition_id_tensor else None
    in_names, out_names, out_avals, zero_shapes = [], [], [], []
    for alloc in nc.m.functions[0].allocations:
        if not isinstance(alloc, mybir.MemoryLocationSet):
            continue
        name = alloc.memorylocations[0].name
        if alloc.kind == "ExternalInput":
            if name != partition_name:
                in_names.append(name)
        elif alloc.kind == "ExternalOutput":
            out_names.append(name)
            shape = tuple(alloc.tensor_shape)
            dtype = mybir.dt.np(alloc.dtype)
            out_avals.append(jax.core.ShapedArray(shape, dtype))
            zero_shapes.append((shape, dtype))
    n_params = len(in_names)
    all_in_names = list(in_names) + list(out_names)
    if partition_name is not None:
        all_in_names.append(partition_name)
    donate_nums = tuple(range(n_params, n_params + len(out_names))) if donate else ()

    def _body(*args):
        operands = list(args)
        if partition_name is not None:
            operands.append(partition_id_tensor())
        return tuple(_bass_exec_p.bind(
            *operands,
            out_avals=tuple(out_avals),
            in_names=tuple(all_in_names),
            out_names=tuple(out_names),
            lowering_input_output_aliases=(),
            sim_require_finite=True,
            sim_require_nnan=True,
            nc=nc,
        ))

    devices = jax.devices()[:n_cores]
    mesh = Mesh(np.asarray(devices), ("core",))
    sharded = jax.jit(
        shard_map(_body, mesh=mesh,
                  in_specs=(PartitionSpec("core"),) * (n_params + len(out_names)),
                  out_specs=(PartitionSpec("core"),) * len(out_names),
                  check_rep=False),
        donate_argnums=donate_nums, keep_unused=True)

    def run(per_core_inputs):
        concat_in = [
            np.concatenate([np.ascontiguousarray(per_core_inputs[c][nm])
                            for c in range(n_cores)], axis=0)
            for nm in in_names
        ]
        concat_zeros = [np.zeros((n_cores * s[0], *s[1:]), d)
                        for s, d in zero_shapes]
        outs = sharded(*concat_in, *concat_zeros)
        return [
            {nm: np.asarray(outs[i]).reshape(n_cores, *out_avals[i].shape)[c]
             for i, nm in enumerate(out_names)}
            for c in range(n_cores)
        ]

    run.sharded = sharded
    run.in_names = in_names
    run.out_names = out_names
    run.out_avals = out_avals
    run.zero_shapes = zero_shapes
    run.mesh = mesh
    return run


_CACHE = {}


def _get_gate_runner():
    if "gate" not in _CACHE:
        _CACHE["gate"] = _build_runner(build_gate_program())
    return _CACHE["gate"]


def _get_ffn_runner(ecap, ucap):
    key = ("ffn", ecap, ucap)
    if key not in _CACHE:
        _CACHE[key] = _build_runner(build_ffn_program(ecap, ucap))
    return _CACHE[key]


# ---------------------------------------------------------------------------
# Host-side input prep (layout pre-arrangement; free wrt HW time)
# ---------------------------------------------------------------------------

def _bias_tiles(b, lam_scaled):
    """[F] -> [128, F//128]; column j = b[j*128:(j+1)*128] (tile-major)."""
    nb = (LAM * b if lam_scaled else b).astype(np.float32)
    return np.ascontiguousarray(nb.reshape(-1, 128).T)


def _sub3(a):
    """[H, W] -> [128, H//128, W]  (h-subtile-major transpose layout)."""
    Hh, W = a.shape
    return np.ascontiguousarray(a.reshape(Hh // 128, 128, W).transpose(1, 0, 2))


def _wi4(a):
    """wi [H, F] -> [128, FCH, HT, F//FCH]."""
    Hh, Ff = a.shape
    return np.ascontiguousarray(
        a.reshape(HT, 128, FCH, Ff // FCH).transpose(1, 2, 0, 3))


def _wo4(a):
    """wo [F, Hout] -> [128, FCH, FPC, Hout]."""
    Ff, Ho = a.shape
    return np.ascontiguousarray(
        a.reshape(FCH, FPC, 128, Ho).transpose(2, 0, 1, 3))


def _gwis2(a):
    """gate-wi shard [H, FS] -> [128, 2, HT, FS//2] (column-halved chunks)."""
    Hh, Fs = a.shape
    return np.ascontiguousarray(
        a.reshape(HT, 128, 2, Fs // 2).transpose(1, 2, 0, 3))


def _gate_cst(gate_bi, gate_wo, gate_bo):
    gbo_adj = (gate_bo.astype(np.float64)
               - LAM_ALPHA * gate_wo.astype(np.float64).sum(0)).astype(np.float32)
    return np.ascontiguousarray(np.concatenate([
        _bias_tiles(gate_bi, True), _bias_tiles(gate_bi, False),
        np.broadcast_to(gbo_adj, (128, H))], axis=1))


def _gate_inputs(xfT_b, gwi_b, gwo_b, cstg):
    """Per-core launch-1 inputs.  xfT_b: [H, N] bf16."""
    return [{
        "xt": _sub3(xfT_b[:, c * TOK:(c + 1) * TOK]),
        "gwi": gwi_b, "gwo": gwo_b, "cst": cstg,
    } for c in range(NCORES)]


def _parse_stats(res1):
    """-> l8 [N, 8] f32, m2 [N] f32 from per-core st outputs."""
    l8 = np.empty((N, E), np.float32)
    m2 = np.empty(N, np.float32)
    for c in range(NCORES):
        stc = res1[c]["st"]                      # [128, TB*10]
        for tcb in range(TB):
            rows = slice(c * TOK + tcb * 128, c * TOK + (tcb + 1) * 128)
            l8[rows] = stc[:, tcb * 10:tcb * 10 + 8]
            m2[rows] = stc[:, tcb * 10 + 9]
    return l8, m2


def kernel(x, gate_wi, gate_bi, gate_wo, gate_bo,
           exp_wi, exp_bi, exp_wo, exp_bo):
    import ml_dtypes
    _bf = ml_dtypes.bfloat16
    x = np.asarray(x, np.float32)
    gate_wi = np.asarray(gate_wi, np.float32)
    gate_bi = np.asarray(gate_bi, np.float32)
    gate_wo = np.asarray(gate_wo, np.float32)
    gate_bo = np.asarray(gate_bo, np.float32)
    exp_wi = np.asarray(exp_wi, np.float32)
    exp_bi = np.asarray(exp_bi, np.float32)
    exp_wo = np.asarray(exp_wo, np.float32)
    exp_bo = np.asarray(exp_bo, np.float32)

    xf = x.reshape(N, H)
    xfT = np.ascontiguousarray(xf.T)             # [H, N]
    xfT_b = xfT.astype(_bf)

    # ---- launch 1: gate + top-2 stats ----
    run1 = _get_gate_runner()
    ins1 = _gate_inputs(xfT_b, _wi4(gate_wi.astype(_bf)),
                        _wo4(gate_wo.astype(_bf)),
                        _gate_cst(gate_bi, gate_wo, gate_bo))
    res1 = run1(ins1)
    l8, m2a = _parse_stats(res1)

    # ---- candidate selection (superset, tau-margin) ----
    cand = l8 >= (m2a[:, None] - TAU)                    # [N, E]
    cand_tok = np.nonzero(cand.any(1))[0]
    nu = len(cand_tok)
    if nu == 0:
        return np.zeros((B, S, H), np.float32)
    ucap = 128
    while nu > ucap:
        ucap *= 2
    expert_rows = [np.nonzero(cand[:, e])[0] for e in range(E)]
    maxc = max((len(r) for r in expert_rows), default=1)
    ecap = 32
    while maxc > ecap:
        ecap *= 2

    # ---- launch 2: expert FFNs + exact gate recompute ----
    run2 = _get_ffn_runner(ecap, ucap)
    EC = max(ecap, 128)
    xut = np.zeros((H, ucap), np.float32)
    xut[:, :nu] = xfT[:, cand_tok]
    xut3 = _sub3(xut)
    ins2 = []
    for c in range(NCORES):
        rows = expert_rows[c]
        xct = np.zeros((H, EC), _bf)
        xct[:, :len(rows)] = xfT_b[:, rows]
        boa_c = (exp_bo[c].astype(np.float64)
                 - LAM_ALPHA * exp_wo[c].astype(np.float64).sum(0)).astype(np.float32)
        sl = slice(c * FS, (c + 1) * FS)
        cstf = np.ascontiguousarray(np.concatenate([
            _bias_tiles(exp_bi[c], True), _bias_tiles(exp_bi[c], False),
            np.broadcast_to(boa_c, (128, H)),
            _bias_tiles(LAM * gate_bi[sl], False),
            _bias_tiles(gate_bi[sl], False)], axis=1))
        ins2.append({
            "xct": _sub3(xct),
            "wi": _wi4(exp_wi[c].astype(_bf)),
            "wo": _wo4(exp_wo[c].astype(_bf)),
            "cst": cstf,
            "xut": xut3,
            "gwis": _gwis2(np.ascontiguousarray(gate_wi[:, sl])),
            "gwos": _sub3(np.ascontiguousarray(gate_wo[sl, :])),
        })
    res2 = run2(ins2)

    # ---- exact logits for candidate tokens, exact top-2 + weights ----
    gsum = np.zeros((ucap, H), np.float64)
    for c in range(NCORES):
        gsum += res2[c]["gp"].astype(np.float64)
    logits_u = gsum[:nu] + (gate_bo.astype(np.float64)
                            - LAM_ALPHA * gate_wo.astype(np.float64).sum(0))
    part = np.partition(logits_u, (H - 2, H - 1), axis=1)
    m1x, m2x = part[:, -1], part[:, -2]
    denom = m1x + m2x
    upos = np.full(N, -1, np.int64)
    upos[cand_tok] = np.arange(nu)

    out = np.zeros((N, H), np.float64)
    for e in range(E):
        rows = expert_rows[e]
        if len(rows) == 0:
            continue
        ye = res2[e]["y"][:len(rows)].astype(np.float64)
        pu = upos[rows]
        le = logits_u[pu, e]
        routed = le >= m2x[pu]
        wgt = np.where(routed, le / denom[pu], 0.0)
        out[rows] += wgt[:, None] * ye
    return out.reshape(B, S, H).astype(np.float32)


# revision 58
# speedup vs baseline: 1.4121x; 1.0955x over previous
"""Trainium2 Bass kernel for nn_MoELayer_5566277616585.

MoE layer with a quirk: the gate FFN outputs H=1024 logits, top-2 indices
>= E=8 are dropped, so ~98% of tokens route nowhere.  Strategy:

Launch 1 (bf16, fast): token-parallel gate FFN on 8 cores (512 tokens
  each; bf16 x@wi and h@wo).  Returns per-token top-8 logit slice +
  (max, 2nd max).  Approx error ~4.5e-2 on device, so launch 1 only
  *selects candidates* with a tau-margin superset.  mm2 runs
  token-block-outer, halves sequential, with per-half top-2 stats
  (m2 = max(min(m1a,m1b), m2a, m2b)) emitted inline so the stats tail
  overlaps remaining matmuls.
Launch 2: expert-parallel bf16 FFN over candidate tokens (core c =
  expert c) PLUS an F-sharded exact fp32 recompute of the gate logits for
  all candidate tokens (1/8 of ffn dim per core).  The recompute streams
  on the Act DMA queue and its mm2 is interleaved between the expert's
  mm1/mm2 so the PE stays fed while 17MB of expert weights stream on the
  SP queue.  Host combines: exact top-2 membership + exact weights from
  the recomputed logits.  (The recompute must stay fp32: the data's
  minimum real-expert decision gap is 5.9e-4, so f32r/bf16 would risk a
  membership flip.)

SELU is composed as  selu(z) = Relu(lam*z + lam*b) + lam*alpha*min(exp(z+b),1)
  - lam*alpha, with the constant -lam*alpha folded into the *output* bias
  via  bo_adj = bo - lam*alpha*colsum(wo).

All weight/activation tensors are pre-arranged on the host into
[128, chunk, subtile, cols] layouts so each launch needs only ~15 large
DMAs (HWDGE descriptor-generation overhead is ~625ns per DMA and was the
previous bottleneck at ~150 DMAs/launch).
"""

import numpy as np

import jax
from jax.experimental.shard_map import shard_map
from jax.sharding import Mesh, PartitionSpec

import concourse.bass as bass
import concourse.tile as tile
from concourse import bacc, mybir
from concourse.bass2jax import _bass_exec_p, install_neuronx_cc_hook, partition_id_tensor

F32 = mybir.dt.float32
F32R = mybir.dt.float32r
BF16 = mybir.dt.bfloat16
AX = mybir.AxisListType
OP = mybir.AluOpType
ACT = mybir.ActivationFunctionType

B, S, H, F, E = 2, 2048, 1024, 4096, 8
N = B * S              # 4096 tokens
NCORES = 8
TOK = N // NCORES      # 512 tokens per core in launch 1
TB = TOK // 128        # token blocks per core
LAM = 1.0507009873554805
ALPHA = 1.6732632423543772
LAM_ALPHA = LAM * ALPHA
TAU = 0.4              # candidate margin (bf16 l8 + fp8 m2-estimate errors)

HT = H // 128          # 8 h-subtiles (contraction tiles for mm1)
FT = F // 128          # 32 f-subtiles (contraction tiles for mm2)
FCH = 8                # wi/wo DMA chunks; each covers F//FCH = 512 f-cols
FPC = FT // FCH        # f-subtiles per chunk = 4
FS = F // NCORES       # 512: gate ffn shard per core in launch 2
FST = FS // 128        # 4 f-subtiles in the gate shard
NP = FT // 2           # 16 f-subtile pairs for the DoubleRow fp8 mm2
F8 = mybir.dt.float8e4
WO8_SCALE = 64.0       # gwo is scaled by this before fp8 cast


def _ffn_bf16(nc, pools, xsl, wi_ap, wo_ap, lbi, bi, boa, out_sb, ntok,
              on_block=None, pre_mm2=None, mm2_fill=None):
    """out_sb[:ntok, :H] = selu'(x@wi+bi) @ wo + boa, all-bf16 matmuls.

    xsl(ht): returns the SBUF AP [128, ntok] of x^T for h-subtile ht.
    wi_ap: DRAM [128, FCH, HT, F//FCH] bf16.  wo_ap: DRAM [128, FCH, FPC, H].
    lbi/bi: SBUF [128, FT] per-partition biases (lam*b, b).
    boa: SBUF [128, H] adjusted output bias (row-replicated) or None.
    mm2 runs token-block-outer; after block tc's output is written,
    on_block(tc) is invoked so the caller can emit dependent work early.
    mm2_fill: list of thunks emitted at mm2 wo-chunk boundaries (PE filler
    while the next weight chunk streams).
    """
    wipool, wopool, ps1, ps2, tpool, hpool = pools
    FW = F // FCH
    hs = []
    for fc in range(FCH):
        w = wipool.tile([128, HT * FW], BF16, tag="wi")
        nc.sync.dma_start(w[:], wi_ap[:, fc])
        for f4 in range(FPC):
            ft = fc * FPC + f4
            ps = ps1.tile([128, ntok], F32)
            for ht in range(HT):
                o = ht * FW + f4 * 128
                nc.tensor.matmul(ps[:], w[:, o:o + 128], xsl(ht),
                                 start=(ht == 0), stop=(ht == HT - 1))
            # selu'(z) = relu(lam*z + lam*b) + lam*alpha*min(exp(z+b), 1)
            r = tpool.tile([128, ntok], F32, tag="selu_r")
            nc.scalar.activation(r[:], ps[:], ACT.Relu,
                                 bias=lbi[:, ft:ft + 1], scale=LAM)
            t = tpool.tile([128, ntok], F32, tag="selu_t")
            nc.scalar.activation(t[:], ps[:], ACT.Exp,
                                 bias=bi[:, ft:ft + 1], scale=1.0)
            e2 = tpool.tile([128, ntok], F32, tag="selu_e")
            nc.vector.tensor_scalar(e2[:], t[:], 1.0, LAM_ALPHA,
                                    op0=OP.min, op1=OP.mult)
            h = hpool.tile([128, ntok], BF16, tag="h")
            nc.vector.tensor_tensor(h[:], r[:], e2[:], op=OP.add)
            hs.append(h)
    if pre_mm2 is not None:
        pre_mm2()
    # --- matmul 2: out[tok, H] = h @ wo (+ boa), token-block-outer ---
    tchunks = (ntok + 127) // 128
    wos = {}
    for tc in range(tchunks):
        tn = min(128, ntok - tc * 128)
        if on_block is None:
            # halves interleaved per ft (best for streaming wo chunks)
            fills = list(mm2_fill or [])
            pss = [ps2.tile([tn, 512], F32, tag="pss", name=f"pss{tc}_{hc}")
                   for hc in range(2)]
            for ft in range(FT):
                fc, f8 = ft // FPC, ft % FPC
                if tc == 0 and f8 == 0:
                    wt = wopool.tile([128, FPC * H], BF16, tag="wo")
                    nc.sync.dma_start(wt[:], wo_ap[:, fc])
                    wos[fc] = wt
                h_sl = hs[ft][:, tc * 128:tc * 128 + tn]
                for hc in range(2):
                    wo_sl = wos[fc][:, f8 * H + hc * 512:f8 * H + hc * 512 + 512]
                    nc.tensor.matmul(pss[hc][:], h_sl, wo_sl,
                                     start=(ft == 0), stop=(ft == FT - 1))
                if tc == 0 and f8 == FPC - 1 and fills:
                    fills.pop(0)()
            for f in fills:
                f()
            for hc in range(2):
                dst = out_sb[tc * 128:tc * 128 + tn, hc * 512:hc * 512 + 512]
                if boa is not None:
                    nc.vector.tensor_add(dst, pss[hc][:],
                                         boa[:tn, hc * 512:hc * 512 + 512])
                else:
                    nc.vector.tensor_copy(dst, pss[hc][:])
        else:
            # halves sequential so the first half's dependents (on_block)
            # overlap the second half's matmuls
            for hc in range(2):
                psh = ps2.tile([tn, 512], F32, tag="pss",
                               name=f"pss{tc}_{hc}")
                for ft in range(FT):
                    fc, f8 = ft // FPC, ft % FPC
                    if tc == 0 and hc == 0 and f8 == 0:
                        wt = wopool.tile([128, FPC * H], BF16, tag="wo")
                        nc.sync.dma_start(wt[:], wo_ap[:, fc])
                        wos[fc] = wt
                    h_sl = hs[ft][:, tc * 128:tc * 128 + tn]
                    wo_sl = wos[fc][:, f8 * H + hc * 512:f8 * H + hc * 512 + 512]
                    nc.tensor.matmul(psh[:], h_sl, wo_sl,
                                     start=(ft == 0), stop=(ft == FT - 1))
                dst = out_sb[tc * 128:tc * 128 + tn, hc * 512:hc * 512 + 512]
                if boa is not None:
                    nc.vector.tensor_add(dst, psh[:],
                                         boa[:tn, hc * 512:hc * 512 + 512])
                else:
                    nc.vector.tensor_copy(dst, psh[:])
                on_block(tc, hc)
    return hs


class _RowView:
    """Token-major [ntok, hout] view over a list of [128, hout] tiles."""

    def __init__(self, tiles):
        self.tiles = tiles

    def __getitem__(self, idx):
        tokslice, hslice = idx
        tc0 = tokslice.start // 128
        return self.tiles[tc0][0:tokslice.stop - tokslice.start, hslice]


def build_gate_program(repeat=1):
    """Launch 1: gate FFN + top-2 stats for 512 tokens/core.

    mm1 bf16; l8 (first 8 logit cols) via a small bf16 matmul on h;
    m2-estimate over all 1024 cols via fp8e4 DoubleRow mm2 (2x PE rate).
    """
    nc = bacc.Bacc("TRN2", target_bir_lowering=False, debug=False,
                   num_devices=NCORES)
    xt = nc.dram_tensor("xt", [128, HT, TOK], BF16, kind="ExternalInput").ap()
    gwi = nc.dram_tensor("gwi", [128, FCH, HT, F // FCH], BF16,
                         kind="ExternalInput").ap()
    gwo8 = nc.dram_tensor("gwo8", [128, NP, 2, H], F8,
                          kind="ExternalInput").ap()
    wo8b = nc.dram_tensor("wo8b", [128, FT * E], BF16,
                          kind="ExternalInput").ap()
    cst = nc.dram_tensor("cst", [128, 2 * FT + H + 1], F32,
                         kind="ExternalInput").ap()
    l8d = nc.dram_tensor("l8d", [E, TOK], F32, kind="ExternalOutput").ap()
    st = nc.dram_tensor("st", [128, TB * 2], F32, kind="ExternalOutput").ap()

    DR = mybir.MatmulPerfMode.DoubleRow
    FW = F // FCH
    with tile.TileContext(nc) as tc:
        import contextlib
        with contextlib.ExitStack() as ctx:
            xpool = ctx.enter_context(tc.tile_pool(name="x", bufs=1))
            cpool = ctx.enter_context(tc.tile_pool(name="consts", bufs=1))
            wipool = ctx.enter_context(tc.tile_pool(name="wi", bufs=3))
            wopool = ctx.enter_context(tc.tile_pool(name="wo", bufs=4))
            ps1 = ctx.enter_context(tc.tile_pool(name="ps1", bufs=4, space="PSUM"))
            ps2 = ctx.enter_context(tc.tile_pool(name="ps2", bufs=3, space="PSUM"))
            psl = ctx.enter_context(tc.tile_pool(name="psl", bufs=1, space="PSUM"))
            tpool = ctx.enter_context(tc.tile_pool(name="tmp", bufs=3))
            hpool = ctx.enter_context(tc.tile_pool(name="h", bufs=FT))
            h8pool = ctx.enter_context(tc.tile_pool(name="h8", bufs=NP))
            zpool = ctx.enter_context(tc.tile_pool(name="z", bufs=TB))
            spool = ctx.enter_context(tc.tile_pool(name="small", bufs=8))
            epool = ctx.enter_context(tc.tile_pool(name="eq", bufs=2))

            def body(_i=None):
                # x^T in two tiles so mm1's first matmuls start after half
                xt_a = xpool.tile([128, HT // 2 * TOK], BF16, tag="xta")
                nc.scalar.dma_start(xt_a[:], xt[:, 0:HT // 2, :])
                xt_b = xpool.tile([128, HT // 2 * TOK], BF16, tag="xtb")
                nc.scalar.dma_start(xt_b[:], xt[:, HT // 2:, :])

                def xsl(ht):
                    t = xt_a if ht < HT // 2 else xt_b
                    o = (ht % (HT // 2)) * TOK
                    return t[:, o:o + TOK]

                cst_t = cpool.tile([128, 2 * FT + H + 1], F32, tag="cst")
                nc.scalar.dma_start(cst_t[:], cst[:, :])
                lbi = cst_t[:, 0:FT]
                bi = cst_t[:, FT:2 * FT]
                boa = cst_t[:, 2 * FT:2 * FT + H]
                w8b_t = cpool.tile([128, FT * E], BF16, tag="w8b")
                nc.scalar.dma_start(w8b_t[:], wo8b[:, :])

                # ---- mm1 (bf16) + selu; h kept in bf16 and fp8-pair form --
                hs, h8s = [], []
                for fc in range(FCH):
                    w = wipool.tile([128, HT * FW], BF16, tag="wi")
                    nc.sync.dma_start(w[:], gwi[:, fc])
                    for f4 in range(FPC):
                        ft = fc * FPC + f4
                        ps = ps1.tile([128, TOK], F32)
                        for ht in range(HT):
                            o = ht * FW + f4 * 128
                            nc.tensor.matmul(ps[:], w[:, o:o + 128], xsl(ht),
                                             start=(ht == 0),
                                             stop=(ht == HT - 1))
                        r = tpool.tile([128, TOK], F32, tag="selu_r")
                        nc.scalar.activation(r[:], ps[:], ACT.Relu,
                                             bias=lbi[:, ft:ft + 1], scale=LAM)
                        t = tpool.tile([128, TOK], F32, tag="selu_t")
                        nc.scalar.activation(t[:], ps[:], ACT.Exp,
                                             bias=bi[:, ft:ft + 1], scale=1.0)
                        e2 = tpool.tile([128, TOK], F32, tag="selu_e")
                        nc.vector.tensor_scalar(e2[:], t[:], 1.0, LAM_ALPHA,
                                                op0=OP.min, op1=OP.mult)
                        h = hpool.tile([128, TOK], BF16, tag="h")
                        nc.vector.tensor_tensor(h[:], r[:], e2[:], op=OP.add)
                        hs.append(h)
                        if ft % 2 == 0:
                            h8s.append(h8pool.tile([128, 2, TOK], F8,
                                                   tag="h8",
                                                   name=f"h8_{ft // 2}"))
                        nc.vector.tensor_copy(h8s[ft // 2][:, ft % 2, :],
                                              h[:])

                # ---- l8: first 8 logit cols, bf16-accurate ----
                pl = psl.tile([E, TOK], F32, tag="pl")
                for ft in range(FT):
                    nc.tensor.matmul(pl[:], w8b_t[:, ft * E:(ft + 1) * E],
                                     hs[ft][:, :],
                                     start=(ft == 0), stop=(ft == FT - 1))
                l8sb = spool.tile([E, TOK], F32, tag="l8sb")
                nc.vector.tensor_scalar(
                    l8sb[:], pl[:], cst_t[0:E, 2 * FT + H:2 * FT + H + 1],
                    None, op0=OP.add)
                nc.scalar.dma_start(l8d[:, :], l8sb[:])

                # ---- m2-estimate mm2: fp8 DoubleRow over all 1024 cols ----
                wo8s = {}
                zs = [zpool.tile([128, H], F32, tag="z", name=f"z{i}")
                      for i in range(TB)]
                stt = spool.tile([128, TB * 2], F32, tag="stt")
                halves = {}

                def stats_half(tcb, hc):
                    z = zs[tcb]
                    zh = z[:, hc * 512:(hc + 1) * 512]
                    m1h = spool.tile([128, 1], F32, tag="m1h",
                                     name=f"m1h{tcb}_{hc}")
                    nc.vector.tensor_reduce(m1h[:], zh, AX.X, OP.max)
                    eq = epool.tile([128, 512], F32, tag="eq")
                    nc.vector.tensor_scalar(eq[:], zh, m1h[:, 0:1], None,
                                            op0=OP.is_equal)
                    msk = epool.tile([128, 512], F32, tag="msk")
                    nc.vector.scalar_tensor_tensor(msk[:], eq[:], -1e30, zh,
                                                   op0=OP.mult, op1=OP.add)
                    m2h = spool.tile([128, 1], F32, tag="m2h",
                                     name=f"m2h{tcb}_{hc}")
                    nc.vector.tensor_reduce(m2h[:], msk[:], AX.X, OP.max)
                    halves[(tcb, hc)] = (m1h, m2h)
                    if hc == 1:
                        m1a, m2a = halves[(tcb, 0)]
                        m1b, m2b = halves[(tcb, 1)]
                        u = spool.tile([128, 1], F32, tag="u", name=f"u{tcb}")
                        nc.vector.tensor_tensor(u[:], m1a[:], m1b[:],
                                                op=OP.min)
                        v = spool.tile([128, 1], F32, tag="v", name=f"v{tcb}")
                        nc.vector.tensor_tensor(v[:], m2a[:], m2b[:],
                                                op=OP.max)
                        o = tcb * 2
                        nc.vector.tensor_tensor(stt[:, o:o + 1],
                                                m1a[:], m1b[:], op=OP.max)
                        nc.vector.tensor_tensor(stt[:, o + 1:o + 2],
                                                u[:], v[:], op=OP.max)
                        nc.sync.dma_start(st[:, o:o + 2], stt[:, o:o + 2])

                for tcb in range(TB):
                    for hc in range(2):
                        ps = ps2.tile([128, 512], F32, tag="pss",
                                      name=f"pss{tcb}_{hc}")
                        for j in range(NP):
                            if tcb == 0 and hc == 0 and j % 4 == 0:
                                wt = wopool.tile([128, 4, 2, H], F8,
                                                 tag="wo8")
                                nc.sync.dma_start(
                                    wt[:], gwo8[:, j:j + 4, :, :])
                                wo8s[j // 4] = wt
                            lhsT = h8s[j][:, :, tcb * 128:(tcb + 1) * 128]
                            rhs = wo8s[j // 4][:, j % 4, :,
                                               hc * 512:hc * 512 + 512]
                            nc.tensor.matmul(ps[:], lhsT, rhs,
                                             start=(j == 0),
                                             stop=(j == NP - 1),
                                             perf_mode=DR)
                        nc.vector.scalar_tensor_tensor(
                            zs[tcb][:, hc * 512:hc * 512 + 512], ps[:],
                            1.0 / WO8_SCALE,
                            boa[:, hc * 512:hc * 512 + 512],
                            op0=OP.mult, op1=OP.add)
                        stats_half(tcb, hc)

            if repeat > 1:
                with tc.For_i(0, repeat, 1):
                    body()
            else:
                body()

    nc.compile()
    return nc


def build_ffn_program(ecap, ucap=128, repeat=1, parts="both"):
    """Launch 2: expert FFN on candidates (bf16) + exact gate F-shard (fp32)."""
    assert ecap <= 128 and ucap <= 512
    EC = max(ecap, 128)
    UB = (ucap + 127) // 128
    nc = bacc.Bacc("TRN2", target_bir_lowering=False, debug=False,
                   num_devices=NCORES)

    xct = nc.dram_tensor("xct", [128, HT, EC], BF16, kind="ExternalInput").ap()
    wi = nc.dram_tensor("wi", [128, FCH, HT, F // FCH], BF16,
                        kind="ExternalInput").ap()
    wo = nc.dram_tensor("wo", [128, FCH, FPC, H], BF16,
                        kind="ExternalInput").ap()
    cst = nc.dram_tensor("cst", [128, 2 * FT + H + 2 * FST], F32,
                         kind="ExternalInput").ap()
    xut = nc.dram_tensor("xut", [128, HT, ucap], F32, kind="ExternalInput").ap()
    gwis = nc.dram_tensor("gwis", [128, 2, HT, FS // 2], F32,
                          kind="ExternalInput").ap()
    gwos = nc.dram_tensor("gwos", [128, FST, H], F32, kind="ExternalInput").ap()
    y = nc.dram_tensor("y", [ecap, H], F32, kind="ExternalOutput").ap()
    gp = nc.dram_tensor("gp", [ucap, H], F32, kind="ExternalOutput").ap()

    with tile.TileContext(nc) as tc:
        import contextlib
        with contextlib.ExitStack() as ctx:
            xpool = ctx.enter_context(tc.tile_pool(name="x", bufs=2))
            cpool = ctx.enter_context(tc.tile_pool(name="consts", bufs=1))
            wipool = ctx.enter_context(tc.tile_pool(name="wi", bufs=2))
            wopool = ctx.enter_context(tc.tile_pool(name="wo", bufs=FCH))
            ps1 = ctx.enter_context(tc.tile_pool(name="ps1", bufs=3, space="PSUM"))
            ps2 = ctx.enter_context(tc.tile_pool(name="ps2", bufs=2, space="PSUM"))
            ps3 = ctx.enter_context(tc.tile_pool(name="ps3", bufs=2, space="PSUM"))
            tpool = ctx.enter_context(tc.tile_pool(name="tmp", bufs=3))
            hpool = ctx.enter_context(tc.tile_pool(name="h", bufs=FT + FST))
            opool = ctx.enter_context(tc.tile_pool(name="outs", bufs=2))

            def body(_i=None):
                do_expert = parts in ("both", "expert")
                do_shard = parts in ("both", "shard")
                cst_t = cpool.tile([128, 2 * FT + H + 2 * FST], F32, tag="cst")
                nc.sync.dma_start(cst_t[:], cst[:, :])
                lbi = cst_t[:, 0:FT]
                bi = cst_t[:, FT:2 * FT]
                boa = cst_t[:, 2 * FT:2 * FT + H]
                lgbis = cst_t[:, 2 * FT + H:2 * FT + H + FST]
                gbis = cst_t[:, 2 * FT + H + FST:2 * FT + H + 2 * FST]

                # gate F-shard exact fp32 recompute, emitted FIRST: its
                # inputs stream on the Act DMA queue while the (much larger)
                # expert weights stream concurrently on the SP queue, so the
                # recompute's matmuls fill the PE while expert weights load.
                if do_shard:
                    xut_t = xpool.tile([128, HT * ucap], F32, tag="xut")
                    nc.scalar.dma_start(xut_t[:], xut[:, :, :])
                    # gwis arrives in 2 chunks (cols split) so mm1 starts early
                    gwis_t = cpool.tile([128, HT * FS], F32, tag="gwis")
                    nc.scalar.dma_start(gwis_t[:, 0:HT * FS // 2],
                                        gwis[:, 0, :, :])
                    nc.scalar.dma_start(gwis_t[:, HT * FS // 2:],
                                        gwis[:, 1, :, :])
                    gwos_t = cpool.tile([128, FST * H], F32, tag="gwos")
                    nc.scalar.dma_start(gwos_t[:], gwos[:, :, :])
                    hus = []
                    for fst in range(FST):
                        ps = ps1.tile([128, ucap], F32)
                        ch, fl = fst // (FST // 2), fst % (FST // 2)
                        for ht in range(HT):
                            o = (ch * HT + ht) * (FS // 2) + fl * 128
                            nc.tensor.matmul(
                                ps[:], gwis_t[:, o:o + 128],
                                xut_t[:, ht * ucap:(ht + 1) * ucap],
                                start=(ht == 0), stop=(ht == HT - 1))
                        r = tpool.tile([128, ucap], F32, tag="selu_r")
                        nc.vector.tensor_scalar(r[:], ps[:],
                                                lgbis[:, fst:fst + 1], 0.0,
                                                op0=OP.add, op1=OP.max)
                        t = tpool.tile([128, ucap], F32, tag="selu_t")
                        nc.scalar.activation(t[:], ps[:], ACT.Exp,
                                             bias=gbis[:, fst:fst + 1],
                                             scale=1.0)
                        e2 = tpool.tile([128, ucap], F32, tag="selu_e")
                        nc.vector.tensor_scalar(e2[:], t[:], 1.0, LAM_ALPHA,
                                                op0=OP.min, op1=OP.mult)
                        hu = hpool.tile([128, ucap], F32, tag="hu")
                        nc.vector.scalar_tensor_tensor(hu[:], r[:], LAM, e2[:],
                                                       op0=OP.mult, op1=OP.add)
                        hus.append(hu)
                    # rec-mm2 split into 4 pieces, used as PE filler at the
                    # expert mm2's wo-chunk boundaries
                    gps = [opool.tile([min(128, ucap - 128 * i), H], F32,
                                      tag="gp", name=f"gp{i}")
                           for i in range(UB)]
                    pssus = {}

                    def rec_piece(hc, half):
                        def f():
                            if half == 0:
                                pssus[hc] = [
                                    ps3.tile([min(128, ucap - 128 * i), 512],
                                             F32, tag="pssu",
                                             name=f"pssu{hc}_{i}")
                                    for i in range(UB)]
                            pssu = pssus[hc]
                            for fst in ((0, 1) if half == 0 else (2, 3)):
                                for i in range(UB):
                                    un = min(128, ucap - 128 * i)
                                    o = fst * H + hc * 512
                                    nc.tensor.matmul(
                                        pssu[i][:],
                                        hus[fst][:, i * 128:i * 128 + un],
                                        gwos_t[:, o:o + 512],
                                        start=(fst == 0),
                                        stop=(fst == FST - 1))
                            if half == 1:
                                for i in range(UB):
                                    nc.vector.tensor_copy(
                                        gps[i][:, hc * 512:hc * 512 + 512],
                                        pssu[i][:])
                                if hc == 1:
                                    for i in range(UB):
                                        un = min(128, ucap - 128 * i)
                                        nc.scalar.dma_start(
                                            gp[128 * i:128 * i + un, :],
                                            gps[i][:])
                        return f

                    rec_pieces = [rec_piece(0, 0), rec_piece(0, 1),
                                  rec_piece(1, 0), rec_piece(1, 1)]
                else:
                    rec_pieces = None

                if do_expert:
                    xct_t = xpool.tile([128, HT * EC], BF16, tag="xct")
                    nc.sync.dma_start(xct_t[:], xct[:, :, :])

                    def xsl_e(ht):
                        return xct_t[:, ht * EC:ht * EC + ecap]

                    y_sb = opool.tile([ecap, H], F32, tag="y")
                    _ffn_bf16(nc, (wipool, wopool, ps1, ps2, tpool, hpool),
                              xsl_e, wi, wo, lbi, bi, boa, y_sb[:, :], ecap,
                              mm2_fill=rec_pieces)
                    nc.sync.dma_start(y[:, :], y_sb[:, :])
                elif rec_pieces is not None:
                    for f in rec_pieces:
                        f()

            if repeat > 1:
                with tc.For_i(0, repeat, 1):
                    body()
            else:
                body()

    nc.compile()
    return nc


# ---------------------------------------------------------------------------
# SPMD runner (cached jit), mirrors concourse.bass2jax.run_bass_via_pjrt
# ---------------------------------------------------------------------------

def _build_runner(nc, n_cores=NCORES, donate=True):
    install_neuronx_cc_hook()
    partition_name = nc.partition_id_tensor.name if nc.partition_id_tensor else None
    in_names, out_names, out_avals, zero_shapes = [], [], [], []
    for alloc in nc.m.functions[0].allocations:
        if not isinstance(alloc, mybir.MemoryLocationSet):
            continue
        name = alloc.memorylocations[0].name
        if alloc.kind == "ExternalInput":
            if name != partition_name:
                in_names.append(name)
        elif alloc.kind == "ExternalOutput":
            out_names.append(name)
            shape = tuple(alloc.tensor_shape)
            dtype = mybir.dt.np(alloc.dtype)
            out_avals.append(jax.core.ShapedArray(shape, dtype))
            zero_shapes.append((shape, dtype))
    n_params = len(in_names)
    all_in_names = list(in_names) + list(out_names)
    if partition_name is not None:
        all_in_names.append(partition_name)
    donate_nums = tuple(range(n_params, n_params + len(out_names))) if donate else ()

    def _body(*args):
        operands = list(args)
        if partition_name is not None:
            operands.append(partition_id_tensor())
        return tuple(_bass_exec_p.bind(
            *operands,
            out_avals=tuple(out_avals),
            in_names=tuple(all_in_names),
            out_names=tuple(out_names),
            lowering_input_output_aliases=(),
            sim_require_finite=True,
            sim_require_nnan=True,
            nc=nc,
        ))

    devices = jax.devices()[:n_cores]
    mesh = Mesh(np.asarray(devices), ("core",))
    sharded = jax.jit(
        shard_map(_body, mesh=mesh,
                  in_specs=(PartitionSpec("core"),) * (n_params + len(out_names)),
                  out_specs=(PartitionSpec("core"),) * len(out_names),
                  check_rep=False),
        donate_argnums=donate_nums, keep_unused=True)

    def run(per_core_inputs):
        concat_in = [
            np.concatenate([np.ascontiguousarray(per_core_inputs[c][nm])
                            for c in range(n_cores)], axis=0)
            for nm in in_names
        ]
        concat_zeros = [np.zeros((n_cores * s[0], *s[1:]), d)
                        for s, d in zero_shapes]
        outs = sharded(*concat_in, *concat_zeros)
        return [
            {nm: np.asarray(outs[i]).reshape(n_cores, *out_avals[i].shape)[c]
             for i, nm in enumerate(out_names)}
            for c in range(n_cores)
        ]

    run.sharded = sharded
    run.in_names = in_names
    run.out_names = out_names
    run.out_avals = out_avals
    run.zero_shapes = zero_shapes
    run.mesh = mesh
    return run


_CACHE = {}


def _get_gate_runner():
    if "gate" not in _CACHE:
        _CACHE["gate"] = _build_runner(build_gate_program())
    return _CACHE["gate"]


def _get_ffn_runner(ecap, ucap):
    key = ("ffn", ecap, ucap)
    if key not in _CACHE:
        _CACHE[key] = _build_runner(build_ffn_program(ecap, ucap))
    return _CACHE[key]


# ---------------------------------------------------------------------------
# Host-side input prep (layout pre-arrangement; free wrt HW time)
# ---------------------------------------------------------------------------

def _bias_tiles(b, lam_scaled):
    """[F] -> [128, F//128]; column j = b[j*128:(j+1)*128] (tile-major)."""
    nb = (LAM * b if lam_scaled else b).astype(np.float32)
    return np.ascontiguousarray(nb.reshape(-1, 128).T)


def _sub3(a):
    """[H, W] -> [128, H//128, W]  (h-subtile-major transpose layout)."""
    Hh, W = a.shape
    return np.ascontiguousarray(a.reshape(Hh // 128, 128, W).transpose(1, 0, 2))


def _wi4(a):
    """wi [H, F] -> [128, FCH, HT, F//FCH]."""
    Hh, Ff = a.shape
    return np.ascontiguousarray(
        a.reshape(HT, 128, FCH, Ff // FCH).transpose(1, 2, 0, 3))


def _wo4(a):
    """wo [F, Hout] -> [128, FCH, FPC, Hout]."""
    Ff, Ho = a.shape
    return np.ascontiguousarray(
        a.reshape(FCH, FPC, 128, Ho).transpose(2, 0, 1, 3))


def _gwis2(a):
    """gate-wi shard [H, FS] -> [128, 2, HT, FS//2] (column-halved chunks)."""
    Hh, Fs = a.shape
    return np.ascontiguousarray(
        a.reshape(HT, 128, 2, Fs // 2).transpose(1, 2, 0, 3))


def _gate_inputs(xfT_b, gate_wi, gate_bi, gate_wo, gate_bo):
    """Per-core launch-1 inputs.  xfT_b: [H, N] bf16; rest f32."""
    import ml_dtypes
    _bf = ml_dtypes.bfloat16
    _f8 = ml_dtypes.float8_e4m3
    gbo_adj = (gate_bo.astype(np.float64)
               - LAM_ALPHA * gate_wo.astype(np.float64).sum(0)).astype(np.float32)
    l8b = np.zeros((128, 1), np.float32)
    l8b[:E, 0] = gbo_adj[:E]
    cstg = np.ascontiguousarray(np.concatenate([
        _bias_tiles(gate_bi, True), _bias_tiles(gate_bi, False),
        np.broadcast_to(gbo_adj, (128, H)), l8b], axis=1))
    gwi_b = _wi4(gate_wi.astype(_bf))
    gwo8 = np.ascontiguousarray(
        (gate_wo * WO8_SCALE).astype(_f8)
        .reshape(NP, 2, 128, H).transpose(2, 0, 1, 3))
    wo8b = np.ascontiguousarray(
        gate_wo[:, :E].astype(_bf).reshape(FT, 128, E).transpose(1, 0, 2))
    return [{
        "xt": _sub3(xfT_b[:, c * TOK:(c + 1) * TOK]),
        "gwi": gwi_b, "gwo8": gwo8, "wo8b": wo8b, "cst": cstg,
    } for c in range(NCORES)]


def _parse_stats(res1):
    """-> l8 [N, 8] f32, m2 [N] f32 from per-core l8d/st outputs."""
    l8 = np.empty((N, E), np.float32)
    m2 = np.empty(N, np.float32)
    for c in range(NCORES):
        l8[c * TOK:(c + 1) * TOK] = res1[c]["l8d"].T    # [E, TOK] -> [TOK, E]
        stc = res1[c]["st"]                             # [128, TB*2]
        for tcb in range(TB):
            rows = slice(c * TOK + tcb * 128, c * TOK + (tcb + 1) * 128)
            m2[rows] = stc[:, tcb * 2 + 1]
    return l8, m2


def kernel(x, gate_wi, gate_bi, gate_wo, gate_bo,
           exp_wi, exp_bi, exp_wo, exp_bo):
    import ml_dtypes
    _bf = ml_dtypes.bfloat16
    x = np.asarray(x, np.float32)
    gate_wi = np.asarray(gate_wi, np.float32)
    gate_bi = np.asarray(gate_bi, np.float32)
    gate_wo = np.asarray(gate_wo, np.float32)
    gate_bo = np.asarray(gate_bo, np.float32)
    exp_wi = np.asarray(exp_wi, np.float32)
    exp_bi = np.asarray(exp_bi, np.float32)
    exp_wo = np.asarray(exp_wo, np.float32)
    exp_bo = np.asarray(exp_bo, np.float32)

    xf = x.reshape(N, H)
    xfT = np.ascontiguousarray(xf.T)             # [H, N]
    xfT_b = xfT.astype(_bf)

    # ---- launch 1: gate + top-2 stats ----
    run1 = _get_gate_runner()
    ins1 = _gate_inputs(xfT_b, gate_wi, gate_bi, gate_wo, gate_bo)
    res1 = run1(ins1)
    l8, m2a = _parse_stats(res1)

    # ---- candidate selection (superset, tau-margin) ----
    cand = l8 >= (m2a[:, None] - TAU)                    # [N, E]
    cand_tok = np.nonzero(cand.any(1))[0]
    nu = len(cand_tok)
    if nu == 0:
        return np.zeros((B, S, H), np.float32)
    ucap = 128
    while nu > ucap:
        ucap *= 2
    expert_rows = [np.nonzero(cand[:, e])[0] for e in range(E)]
    maxc = max((len(r) for r in expert_rows), default=1)
    ecap = 32
    while maxc > ecap:
        ecap *= 2

    # ---- launch 2: expert FFNs + exact gate recompute ----
    run2 = _get_ffn_runner(ecap, ucap)
    EC = max(ecap, 128)
    xut = np.zeros((H, ucap), np.float32)
    xut[:, :nu] = xfT[:, cand_tok]
    xut3 = _sub3(xut)
    ins2 = []
    for c in range(NCORES):
        rows = expert_rows[c]
        xct = np.zeros((H, EC), _bf)
        xct[:, :len(rows)] = xfT_b[:, rows]
        boa_c = (exp_bo[c].astype(np.float64)
                 - LAM_ALPHA * exp_wo[c].astype(np.float64).sum(0)).astype(np.float32)
        sl = slice(c * FS, (c + 1) * FS)
        cstf = np.ascontiguousarray(np.concatenate([
            _bias_tiles(exp_bi[c], True), _bias_tiles(exp_bi[c], False),
            np.broadcast_to(boa_c, (128, H)),
            _bias_tiles(LAM * gate_bi[sl], False),
            _bias_tiles(gate_bi[sl], False)], axis=1))
        ins2.append({
            "xct": _sub3(xct),
            "wi": _wi4(exp_wi[c].astype(_bf)),
            "wo": _wo4(exp_wo[c].astype(_bf)),
            "cst": cstf,
            "xut": xut3,
            "gwis": _gwis2(np.ascontiguousarray(gate_wi[:, sl])),
            "gwos": _sub3(np.ascontiguousarray(gate_wo[sl, :])),
        })
    res2 = run2(ins2)

    # ---- exact logits for candidate tokens, exact top-2 + weights ----
    gsum = np.zeros((ucap, H), np.float64)
    for c in range(NCORES):
        gsum += res2[c]["gp"].astype(np.float64)
    logits_u = gsum[:nu] + (gate_bo.astype(np.float64)
                            - LAM_ALPHA * gate_wo.astype(np.float64).sum(0))
    part = np.partition(logits_u, (H - 2, H - 1), axis=1)
    m1x, m2x = part[:, -1], part[:, -2]
    denom = m1x + m2x
    upos = np.full(N, -1, np.int64)
    upos[cand_tok] = np.arange(nu)

    out = np.zeros((N, H), np.float64)
    for e in range(E):
        rows = expert_rows[e]
        if len(rows) == 0:
            continue
        ye = res2[e]["y"][:len(rows)].astype(np.float64)
        pu = upos[rows]
        le = logits_u[pu, e]
        routed = le >= m2x[pu]
        wgt = np.where(routed, le / denom[pu], 0.0)
        out[rows] += wgt[:, None] * ye
    return out.reshape(B, S, H).astype(np.float32)
